# revision 22
# baseline (speedup 1.0000x reference)
"""Trainium2 Bass kernel for Conv2d_NN (k-NN gather + grouped conv1d).

Shapes (hardcoded): x (32, 32, 96, 96) f32, conv_w (256, 128, 9) f32,
conv_b (256,) f32 -> out (32, 64, 96, 96) f32.

Strategy: data-parallel over batch across 8 NeuronCores (4 batches/core).
Per batch on device (tokens N=2304, features D=128 after host pixel-unshuffle):
  - scores = x2^T @ x2 - 0.5*||x_j||^2 in fp32 on PE (fp32 matmul streams at
    ~2 cyc/col on HW), PSUM-chunked [128,512]x5; the j-dependent nsq term is
    one 2-row fp16 matmul per chunk (host-prepared exact hi+lo split); self
    excluded with a -16384 fp16 diag matmul.  Ranking needs fp32-exact
    scores: fp16/bf16 storage or single-fp16-product dots flip neighbors
    and push rel err to 5e-2..1e-1 (measured on host).
  - ACT evacuates PSUM chunks to an fp32 scores row-block; DVE max8 /
    find_index8 give the top-8 neighbor indices per token.
  - a 2-hop DMA shuffle (SBUF->DRAM->SBUF broadcast) rewraps the [128,8]
    index tile into the 16-partition-wrapped layout, k-major per 512-token
    conv group.
  - dma_gather (SWDGE, transpose mode) pulls neighbor token rows from a
    host-prepared fp16 [N,128] DRAM table straight into conv-rhs layout
    [128 feat, 8*512 tok].  This replaces the baseline's gpsimd ap_gather,
    which ran ~250us per call on the DSP cores and serialized the whole
    kernel (2.15ms); descriptor-generated DMA does the same gather in ~10us
    and overlaps with compute.
  - conv1d = 9 accumulating 128x128 fp16 matmuls per output half (k=0 rhs is
    the fp16 x2 copy, k=1..8 slices of the gathered buffer); ACT adds bias +
    ReLU; DMA writes (b, 256, N) fp32.  fp16 conv adds ~3e-4 rel err.
Host does pixel-unshuffle/shuffle and all dtype prep (fp16 table, nsq
hi/lo rows, fp16 weights).
"""

import sys

for _p in ("/opt/trn_rl_repo",):
    if _p not in sys.path:
        sys.path.insert(0, _p)

import numpy as np

import concourse.bass as bass
import concourse.mybir as mybir
import concourse.tile as tile
from concourse import bacc, bass_utils

# Problem constants
B, C_IN, C_OUT, H, W = 32, 32, 64, 96, 96
S = 2
K = 9
D = C_IN * S * S            # 128
D_OUT = C_OUT * S * S       # 256
N = (H // S) * (W // S)     # 2304
NCORES = 8
BPC = B // NCORES           # 4 batches per core

P = 128                     # partitions / m-tile size
NT = N // P                 # 18 m-tiles
CHUNK = 512                 # psum bank = 512 f32; conv group chunk
SCHUNK = 512                # scores psum chunk = 1 bank (matmul cannot cross banks)
CHUNKS = [(c, min(SCHUNK, N - c)) for c in range(0, N, SCHUNK)]  # 4x512 + 256
NEGBIG = -16384.0           # fp16-exact, dominates any real score
GROUP_TILES = 4             # m-tiles per conv group (512 tokens)

# pipeline lags (in tile slots)
GATHER_LAG = 1              # gather emitted this many slots after its group ends
BSTART_LEAD = 6             # batch-start work emitted this many slots early
CONV_DELAY = 5              # conv emitted this many slots after its group ends

_cache = {}


def _build_kernel(bpc=BPC, nt=NT):
    key = ("nc", bpc, nt)
    if key in _cache:
        return _cache[key], None

    nc = bacc.Bacc(
        "TRN2", target_bir_lowering=False, debug=False, num_swdge_queues=4
    )

    f32 = mybir.dt.float32
    fp16 = mybir.dt.float16
    u16 = mybir.dt.uint16
    i16 = mybir.dt.int16

    n_tok = nt * P

    # groups per batch: (start_tile, n_tiles)
    groups = []
    mt = 0
    while mt < nt:
        gt = min(GROUP_TILES, nt - mt)
        groups.append((mt, gt))
        mt += gt
    widx_w = nt * 64                             # 64 wrapped cols per tile

    # I/O
    x2_d = nc.dram_tensor("x2", [bpc, D, n_tok], f32, kind="ExternalInput")
    x2t_d = nc.dram_tensor("x2t", [bpc, P, (n_tok // P) * P], fp16, kind="ExternalInput")
    xhi_d = nc.dram_tensor("xhi", [bpc, D, n_tok], fp16, kind="ExternalInput")
    nsq2_d = nc.dram_tensor("nsq2", [bpc, 2, n_tok], fp16, kind="ExternalInput")
    wt_d = nc.dram_tensor("wt", [D, K, 2, P], fp16, kind="ExternalInput")
    bias_d = nc.dram_tensor("bias", [P, 2], f32, kind="ExternalInput")
    ones2_d = nc.dram_tensor("ones2", [2, P], fp16, kind="ExternalInput")
    negbig_d = nc.dram_tensor("negbig", [P, P], fp16, kind="ExternalInput")
    ident_d = nc.dram_tensor("ident", [P, P], fp16, kind="ExternalInput")
    out_d = nc.dram_tensor("out", [bpc, D_OUT, n_tok], f32, kind="ExternalOutput")

    T = bpc * nt                                 # total tile slots

    with tile.TileContext(nc) as tc:
        import contextlib

        with contextlib.ExitStack() as ctx:
            const_pool = ctx.enter_context(tc.tile_pool(name="consts", bufs=1))
            x2_pool = ctx.enter_context(tc.tile_pool(name="x2", bufs=2))
            xhi_pool = ctx.enter_context(tc.tile_pool(name="xhi", bufs=2))
            tbl_pool = ctx.enter_context(tc.tile_pool(name="tbl", bufs=2))
            nsq_pool = ctx.enter_context(tc.tile_pool(name="nsq", bufs=2))
            scores_pool = ctx.enter_context(tc.tile_pool(name="scores", bufs=4))
            mx_pool = ctx.enter_context(tc.tile_pool(name="mx", bufs=8))
            widx_pool = ctx.enter_context(tc.tile_pool(name="widx", bufs=2))
            g_pool = ctx.enter_context(tc.tile_pool(name="g", bufs=4))
            outs_pool = ctx.enter_context(tc.tile_pool(name="outs", bufs=3))
            psum_sc_pool = ctx.enter_context(
                tc.tile_pool(name="psums", bufs=5, space="PSUM")
            )
            psum_conv_pool = ctx.enter_context(
                tc.tile_pool(name="psumc", bufs=2, space="PSUM")
            )
            dram_pool = ctx.enter_context(
                tc.tile_pool(name="stage", bufs=8, space="DRAM")
            )

            # constants, loaded once
            wt_s = const_pool.tile([D, K * 2 * P], fp16, tag="wt")
            nc.sync.dma_start(wt_s[:], wt_d.ap().rearrange("d k h c -> d (k h c)"))
            wt_v = wt_s[:].rearrange("d (k h c) -> d k h c", k=K, h=2, c=P)
            bias_s = const_pool.tile([P, 2], f32, tag="bias")
            nc.sync.dma_start(bias_s[:], bias_d.ap())
            ones2_s = const_pool.tile([2, P], fp16, tag="ones2")
            nc.sync.dma_start(ones2_s[:], ones2_d.ap())
            negbig_s = const_pool.tile([P, P], fp16, tag="negbig")
            nc.sync.dma_start(negbig_s[:], negbig_d.ap())
            ident_s = const_pool.tile([P, P], fp16, tag="ident")
            nc.sync.dma_start(ident_s[:], ident_d.ap())

            # per-batch state (rotating pool tiles), keyed by batch
            state = {}

            def emit_bstart(b):
                x2 = x2_pool.tile([D, n_tok], f32, tag="x2")
                nc.scalar.dma_start(x2[:], x2_d.ap()[b])
                xhi = xhi_pool.tile([D, n_tok], fp16, tag="xhi")
                nc.scalar.dma_start(xhi[:], xhi_d.ap()[b])
                nsq2 = nsq_pool.tile([2, n_tok], fp16, tag="nsq2")
                nc.scalar.dma_start(nsq2[:], nsq2_d.ap()[b])
                tbl = tbl_pool.tile([P, n_tok], fp16, tag="tbl")
                nc.scalar.dma_start(tbl[:], x2t_d.ap()[b])
                widx = widx_pool.tile([P, widx_w], i16, tag="widx")
                state[b] = dict(
                    x2=x2, xhi=xhi, nsq2=nsq2, tbl=tbl, widx=widx, g={}
                )

            def emit_scores(b, mt):
                st = state[b]
                x2 = st["x2"]
                nsq2 = st["nsq2"]
                m0 = mt * P
                dc = m0 // SCHUNK
                pscs = []
                for ci, (c0, w) in enumerate(CHUNKS):
                    psc = psum_sc_pool.tile([P, SCHUNK], f32, tag="psc")
                    pscs.append(psc)
                    nc.tensor.matmul(
                        psc[:, :w],
                        lhsT=x2[:, m0 : m0 + P],
                        rhs=x2[:, c0 : c0 + w],
                        start=True, stop=False,
                    )
                # self-exclusion: scores[p, m0+p] += NEGBIG
                d0 = m0 - dc * SCHUNK
                nc.tensor.matmul(
                    pscs[dc][:, d0 : d0 + P],
                    lhsT=negbig_s[:],
                    rhs=ident_s[:],
                    start=False, stop=False,
                )
                # j-dependent -0.5*nsq[j] as one 2-row fp16 matmul per chunk
                for ci, (c0, w) in enumerate(CHUNKS):
                    nc.tensor.matmul(
                        pscs[ci][:, :w],
                        lhsT=ones2_s[:],
                        rhs=nsq2[:, c0 : c0 + w],
                        start=False, stop=True,
                    )
                scores = scores_pool.tile([P, n_tok], f32, tag="scores")
                for ci, (c0, w) in enumerate(CHUNKS):
                    nc.scalar.copy(scores[:, c0 : c0 + w], pscs[ci][:, :w])
                st["scores"] = st.get("scores", {})
                st["scores"][mt] = scores

            def emit_topk(b, mt):
                st = state[b]
                scores = st["scores"].pop(mt)
                mx8 = mx_pool.tile([P, 8], f32, tag="mx8")
                nc.vector.max(out=mx8[:], in_=scores[:])
                midx = mx_pool.tile([P, 8], u16, tag="midx")
                nc.vector.max_index(midx[:], mx8[:], scores[:])
                st["midx"] = st.get("midx", {})
                st["midx"][mt] = midx

            def emit_shuffle(b, mt):
                st = state[b]
                midx = st["midx"].pop(mt)
                wg = st["widx"]
                # hop 1: midx [128,8] -> staging[(r*8 + u)*8 + k] (DRAM);
                # k innermost keeps both hops at 16B-contiguous DMA runs
                stage_t = dram_pool.tile([1, 1024], u16, tag="stage")
                st_dst = stage_t[:].rearrange(
                    "a (r u k) -> a u r k", r=16, u=8, k=8
                ).squeeze(0)
                nc.sync.dma_start(st_dst, midx[:])
                # hop 2: widx[16c+r, mt*64 + c2] = staging[r*64 + c2]
                st_src = (
                    stage_t[:]
                    .rearrange("a (r c2) -> a r c2", r=16, c2=64)
                    .unsqueeze(1)
                    .broadcast_to([1, 8, 16, 64])
                    .bitcast(i16)
                    .squeeze(0)
                )
                nc.sync.dma_start(wg[:, mt * 64 : (mt + 1) * 64], st_src)

            def emit_gathers(b, mt):
                # two 512-idx gathers per tile (1024-descriptor SWDGE carveout
                # limit per instruction), round-robin over the 4 SWDGE queues.
                # idx stream position i = ((u*8 + k)*16 + r) within the tile,
                # i.e. token-half-major; conv rhs un-permutes with a 4D view.
                st = state[b]
                if mt % GROUP_TILES == 0 or mt == groups[-1][0]:
                    g = [gi for gi, (gs, gn) in enumerate(groups) if gs == mt][0]
                    gg = g_pool.tile(
                        [P, 1, GROUP_TILES * 1024], fp16, tag="g"
                    )
                    st["g"][g] = gg
                g = [gi for gi, (gs, gn) in enumerate(groups)
                     if gs <= mt < gs + gn][0]
                tloc = mt - groups[g][0]
                gg = st["g"][g]
                for half in range(2):
                    o = tloc * 1024 + half * 512
                    nc.gpsimd.dma_gather(
                        gg[:, :, o : o + 512],
                        st["tbl"][:],
                        st["widx"][:, mt * 64 + half * 32 : mt * 64 + half * 32 + 32],
                        512,
                        512,
                        D,
                        transpose=True,
                        queue_num=(2 * mt + half) % 4,
                        sbuf_tokens_per_rank=P,
                        sbuf_free_dim_per_rank=2 * D,
                    )

            def emit_conv(b, g):
                st = state[b]
                xhi = st["xhi"]
                gstart, gtiles = groups[g]
                gtok = gtiles * P
                g0 = gstart * P
                gv = st["g"].pop(g)[:, 0, : gtiles * 1024].rearrange(
                    "d (t u k r) -> d t u k r", t=gtiles, u=8, k=8, r=16
                )
                for h in range(2):
                    cp = psum_conv_pool.tile([P, CHUNK], f32, tag="pconv")
                    # k = 0: self columns from the fp16 x2 copy
                    nc.tensor.matmul(
                        cp[:, :gtok],
                        lhsT=wt_v[:, 0, h, :],
                        rhs=xhi[:, g0 : g0 + gtok],
                        start=True, stop=False,
                    )
                    for k in range(1, K):
                        nc.tensor.matmul(
                            cp[:, :gtok],
                            lhsT=wt_v[:, k, h, :],
                            rhs=gv[:, :, :, k - 1, :],
                            start=False, stop=(k == K - 1),
                        )
                    o_s = outs_pool.tile([P, CHUNK], f32, tag="outs")
                    nc.scalar.activation(
                        o_s[:, :gtok], cp[:, :gtok],
                        mybir.ActivationFunctionType.Relu,
                        bias=bias_s[:, h : h + 1],
                    )
                    nc.scalar.dma_start(
                        out_d.ap()[b, h * P : (h + 1) * P, g0 : g0 + gtok],
                        o_s[:, :gtok],
                    )
                if g == len(groups) - 1:
                    state.pop(b, None)

            # ---- software-pipelined emission over flat tile slots ----
            group_end = {gs + gn - 1: gi for gi, (gs, gn) in enumerate(groups)}

            emit_bstart(0)
            for s in range(T + CONV_DELAY + 1):
                if s + BSTART_LEAD < T and (s + BSTART_LEAD) % nt == 0:
                    emit_bstart((s + BSTART_LEAD) // nt)
                if s < T:
                    b, mt = divmod(s, nt)
                    emit_scores(b, mt)
                    emit_topk(b, mt)
                    emit_shuffle(b, mt)
                t_g = s - GATHER_LAG
                if 0 <= t_g < T:
                    b, mt = divmod(t_g, nt)
                    emit_gathers(b, mt)
                t_c = s - CONV_DELAY
                if 0 <= t_c < T:
                    b, mt = divmod(t_c, nt)
                    if mt in group_end:
                        emit_conv(b, group_end[mt])

    nc.compile()
    _cache[key] = nc
    return nc, None


def _host_inputs(x, conv_w, conv_b):
    """Per-batch device inputs + shared per-core constants."""
    x = np.ascontiguousarray(x, dtype=np.float32)
    b = x.shape[0]
    x1 = (
        x.reshape(b, C_IN, H // S, S, W // S, S)
        .transpose(0, 1, 3, 5, 2, 4)
        .reshape(b, D, N)
    )
    xhi = x1.astype(np.float16)
    # SBUF gather table: token t = r*128 + p lives at [part p, r*128 : (r+1)*128]
    x2t = np.ascontiguousarray(
        xhi.transpose(0, 2, 1)
        .reshape(b, N // 128, 128, D)
        .transpose(0, 2, 1, 3)
        .reshape(b, 128, N // 128 * D)
    )
    v = -0.5 * np.einsum("bdn,bdn->bn", x1.astype(np.float64), x1.astype(np.float64))
    nhi = v.astype(np.float16)
    nlo = (v - nhi.astype(np.float64)).astype(np.float16)
    nsq2 = np.stack([nhi, nlo], axis=1)                           # [B, 2, N]
    per_batch = dict(x2=x1, x2t=x2t, xhi=np.ascontiguousarray(xhi), nsq2=nsq2)

    wt = np.ascontiguousarray(
        conv_w.reshape(2, P, D, K).transpose(2, 3, 0, 1), dtype=np.float16
    )  # [D, K, 2, P]
    bias = np.ascontiguousarray(
        conv_b.reshape(2, P).transpose(1, 0), dtype=np.float32
    )  # [P, 2]
    ones2 = np.ones((2, P), dtype=np.float16)
    negbig = (NEGBIG * np.eye(P)).astype(np.float16)
    ident = np.eye(P, dtype=np.float16)
    consts = dict(wt=wt, bias=bias, ones2=ones2, negbig=negbig, ident=ident)
    return per_batch, consts


def kernel(x, conv_w, conv_b):
    nc, _ = _build_kernel()
    per_batch, consts = _host_inputs(x, conv_w, conv_b)
    in_maps = []
    for c in range(NCORES):
        m = dict(consts)
        for k, v in per_batch.items():
            m[k] = np.ascontiguousarray(v[c * BPC : (c + 1) * BPC])
        in_maps.append(m)
    res = bass_utils.run_bass_kernel_spmd(nc, in_maps, core_ids=list(range(NCORES)))
    outs = np.concatenate([r["out"] for r in res.results], axis=0)  # [B, 256, N]
    # pixel shuffle back: channel dim = (co, sy, sx); token = (h, w)
    o = outs.reshape(B, C_OUT, S, S, H // S, W // S)
    o = o.transpose(0, 1, 4, 2, 5, 3).reshape(B, C_OUT, H, W)
    return np.ascontiguousarray(o, dtype=np.float32)


# revision 23
# speedup vs baseline: 1.0934x; 1.0934x over previous
"""Trainium2 Bass kernel for Conv2d_NN (k-NN gather + grouped conv1d).

Shapes (hardcoded): x (32, 32, 96, 96) f32, conv_w (256, 128, 9) f32,
conv_b (256,) f32 -> out (32, 64, 96, 96) f32.

Strategy: data-parallel over batch across 8 NeuronCores (4 batches/core).
Per batch on device (tokens N=2304, features D=128 after host pixel-unshuffle):
  - scores = x2^T @ x2 - 0.5*||x_j||^2 in fp32 on PE (fp32 matmul streams at
    ~2 cyc/col on HW), PSUM-chunked [128,512]x5; the j-dependent nsq term is
    one 2-row fp16 matmul per chunk (host-prepared exact hi+lo split); self
    excluded with a -16384 fp16 diag matmul.  Ranking needs fp32-exact
    scores: fp16/bf16 storage or single-fp16-product dots flip neighbors
    and push rel err to 5e-2..1e-1 (measured on host).
  - ACT evacuates PSUM chunks to an fp32 scores row-block; DVE max8 /
    find_index8 give the top-8 neighbor indices per token.
  - a 2-hop DMA shuffle (SBUF->DRAM->SBUF broadcast) rewraps the [128,8]
    index tile into the 16-partition-wrapped layout, k-major per 512-token
    conv group.
  - dma_gather (SWDGE, transpose mode) pulls neighbor token rows from a
    host-prepared fp16 [N,128] DRAM table straight into conv-rhs layout
    [128 feat, 8*512 tok].  This replaces the baseline's gpsimd ap_gather,
    which ran ~250us per call on the DSP cores and serialized the whole
    kernel (2.15ms); descriptor-generated DMA does the same gather in ~10us
    and overlaps with compute.
  - conv1d = 9 accumulating 128x128 fp16 matmuls per output half (k=0 rhs is
    the fp16 x2 copy, k=1..8 slices of the gathered buffer); ACT adds bias +
    ReLU; DMA writes (b, 256, N) fp32.  fp16 conv adds ~3e-4 rel err.
Host does pixel-unshuffle/shuffle and all dtype prep (fp16 table, nsq
hi/lo rows, fp16 weights).
"""

import sys

for _p in ("/opt/trn_rl_repo",):
    if _p not in sys.path:
        sys.path.insert(0, _p)

import numpy as np

import concourse.bass as bass
import concourse.mybir as mybir
import concourse.tile as tile
from concourse import bacc, bass_utils

# Problem constants
B, C_IN, C_OUT, H, W = 32, 32, 64, 96, 96
S = 2
K = 9
D = C_IN * S * S            # 128
D_OUT = C_OUT * S * S       # 256
N = (H // S) * (W // S)     # 2304
NCORES = 8
BPC = B // NCORES           # 4 batches per core

P = 128                     # partitions / m-tile size
NT = N // P                 # 18 m-tiles
CHUNK = 512                 # psum bank = 512 f32; conv group chunk
SCHUNK = 512                # scores psum chunk = 1 bank (matmul cannot cross banks)
CHUNKS = [(c, min(SCHUNK, N - c)) for c in range(0, N, SCHUNK)]  # 4x512 + 256
NEGBIG = -16384.0           # fp16-exact, dominates any real score
GROUP_TILES = 4             # m-tiles per conv group (512 tokens)

# pipeline lags (in tile slots)
GATHER_LAG = 1              # gather emitted this many slots after its group ends
BSTART_LEAD = 6             # batch-start work emitted this many slots early
CONV_DELAY = 6              # conv emitted this many slots after its group ends

_cache = {}


def _build_kernel(bpc=BPC, nt=NT):
    key = ("nc", bpc, nt)
    if key in _cache:
        return _cache[key], None

    nc = bacc.Bacc(
        "TRN2", target_bir_lowering=False, debug=False, num_swdge_queues=4
    )

    f32 = mybir.dt.float32
    fp16 = mybir.dt.float16
    u16 = mybir.dt.uint16
    i16 = mybir.dt.int16

    n_tok = nt * P

    # groups per batch: (start_tile, n_tiles)
    groups = []
    mt = 0
    while mt < nt:
        gt = min(GROUP_TILES, nt - mt)
        groups.append((mt, gt))
        mt += gt
    widx_w = nt * 64                             # 64 wrapped cols per tile

    # I/O
    x2_d = nc.dram_tensor("x2", [bpc, D, n_tok], f32, kind="ExternalInput")
    x2t_d = nc.dram_tensor("x2t", [bpc, P, (n_tok // P) * P], fp16, kind="ExternalInput")
    xhi_d = nc.dram_tensor("xhi", [bpc, D, n_tok], fp16, kind="ExternalInput")
    nsq2_d = nc.dram_tensor("nsq2", [bpc, 2, n_tok], fp16, kind="ExternalInput")
    wt_d = nc.dram_tensor("wt", [D, K, 2, P], fp16, kind="ExternalInput")
    bias_d = nc.dram_tensor("bias", [P, 2], f32, kind="ExternalInput")
    ones2_d = nc.dram_tensor("ones2", [2, P], fp16, kind="ExternalInput")
    negbig_d = nc.dram_tensor("negbig", [P, P], fp16, kind="ExternalInput")
    ident_d = nc.dram_tensor("ident", [P, P], fp16, kind="ExternalInput")
    out_d = nc.dram_tensor("out", [bpc, D_OUT, n_tok], f32, kind="ExternalOutput")

    T = bpc * nt                                 # total tile slots

    with tile.TileContext(nc) as tc:
        import contextlib

        with contextlib.ExitStack() as ctx:
            const_pool = ctx.enter_context(tc.tile_pool(name="consts", bufs=1))
            x2_pool = ctx.enter_context(tc.tile_pool(name="x2", bufs=2))
            xhi_pool = ctx.enter_context(tc.tile_pool(name="xhi", bufs=2))
            tbl_pool = ctx.enter_context(tc.tile_pool(name="tbl", bufs=2))
            nsq_pool = ctx.enter_context(tc.tile_pool(name="nsq", bufs=2))
            scores_pool = ctx.enter_context(tc.tile_pool(name="scores", bufs=4))
            mx_pool = ctx.enter_context(tc.tile_pool(name="mx", bufs=8))
            widx_pool = ctx.enter_context(tc.tile_pool(name="widx", bufs=2))
            g_pool = ctx.enter_context(tc.tile_pool(name="g", bufs=4))
            outs_pool = ctx.enter_context(tc.tile_pool(name="outs", bufs=3))
            psum_sc_pool = ctx.enter_context(
                tc.tile_pool(name="psums", bufs=5, space="PSUM")
            )
            psum_conv_pool = ctx.enter_context(
                tc.tile_pool(name="psumc", bufs=2, space="PSUM")
            )
            dram_pool = ctx.enter_context(
                tc.tile_pool(name="stage", bufs=8, space="DRAM")
            )

            # constants, loaded once
            wt_s = const_pool.tile([D, K * 2 * P], fp16, tag="wt")
            nc.sync.dma_start(wt_s[:], wt_d.ap().rearrange("d k h c -> d (k h c)"))
            wt_v = wt_s[:].rearrange("d (k h c) -> d k h c", k=K, h=2, c=P)
            bias_s = const_pool.tile([P, 2], f32, tag="bias")
            nc.sync.dma_start(bias_s[:], bias_d.ap())
            ones2_s = const_pool.tile([2, P], fp16, tag="ones2")
            nc.sync.dma_start(ones2_s[:], ones2_d.ap())
            negbig_s = const_pool.tile([P, P], fp16, tag="negbig")
            nc.sync.dma_start(negbig_s[:], negbig_d.ap())
            ident_s = const_pool.tile([P, P], fp16, tag="ident")
            nc.sync.dma_start(ident_s[:], ident_d.ap())

            # per-batch state (rotating pool tiles), keyed by batch
            state = {}

            def emit_bstart(b):
                x2 = x2_pool.tile([D, n_tok], f32, tag="x2")
                nc.scalar.dma_start(x2[:], x2_d.ap()[b])
                xhi = xhi_pool.tile([D, n_tok], fp16, tag="xhi")
                nc.scalar.dma_start(xhi[:], xhi_d.ap()[b])
                nsq2 = nsq_pool.tile([2, n_tok], fp16, tag="nsq2")
                nc.scalar.dma_start(nsq2[:], nsq2_d.ap()[b])
                tbl = tbl_pool.tile([P, n_tok], fp16, tag="tbl")
                nc.scalar.dma_start(tbl[:], x2t_d.ap()[b])
                widx = widx_pool.tile([P, widx_w], i16, tag="widx")
                state[b] = dict(
                    x2=x2, xhi=xhi, nsq2=nsq2, tbl=tbl, widx=widx, g={}
                )

            def emit_scores(b, mt):
                st = state[b]
                x2 = st["x2"]
                nsq2 = st["nsq2"]
                m0 = mt * P
                dc = m0 // SCHUNK
                pscs = []
                for ci, (c0, w) in enumerate(CHUNKS):
                    psc = psum_sc_pool.tile([P, SCHUNK], f32, tag="psc")
                    pscs.append(psc)
                    nc.tensor.matmul(
                        psc[:, :w],
                        lhsT=x2[:, m0 : m0 + P],
                        rhs=x2[:, c0 : c0 + w],
                        start=True, stop=False,
                    )
                # self-exclusion: scores[p, m0+p] += NEGBIG
                d0 = m0 - dc * SCHUNK
                nc.tensor.matmul(
                    pscs[dc][:, d0 : d0 + P],
                    lhsT=negbig_s[:],
                    rhs=ident_s[:],
                    start=False, stop=False,
                )
                # j-dependent -0.5*nsq[j] as one 2-row fp16 matmul per chunk
                for ci, (c0, w) in enumerate(CHUNKS):
                    nc.tensor.matmul(
                        pscs[ci][:, :w],
                        lhsT=ones2_s[:],
                        rhs=nsq2[:, c0 : c0 + w],
                        start=False, stop=True,
                    )
                scores = scores_pool.tile([P, n_tok], f32, tag="scores")
                for ci, (c0, w) in enumerate(CHUNKS):
                    nc.scalar.copy(scores[:, c0 : c0 + w], pscs[ci][:, :w])
                st["scores"] = st.get("scores", {})
                st["scores"][mt] = scores

            def emit_topk(b, mt):
                st = state[b]
                scores = st["scores"].pop(mt)
                mx8 = mx_pool.tile([P, 8], f32, tag="mx8")
                nc.vector.max(out=mx8[:], in_=scores[:])
                midx = mx_pool.tile([P, 8], u16, tag="midx")
                nc.vector.max_index(midx[:], mx8[:], scores[:])
                st["midx"] = st.get("midx", {})
                st["midx"][mt] = midx

            def emit_shuffle(b, mt):
                st = state[b]
                midx = st["midx"].pop(mt)
                wg = st["widx"]
                # hop 1: midx [128,8] -> staging[(r*8 + u)*8 + k] (DRAM);
                # k innermost keeps both hops at 16B-contiguous DMA runs
                stage_t = dram_pool.tile([1, 1024], u16, tag="stage")
                st_dst = stage_t[:].rearrange(
                    "a (r u k) -> a u r k", r=16, u=8, k=8
                ).squeeze(0)
                nc.sync.dma_start(st_dst, midx[:])
                # hop 2: widx[16c+r, mt*64 + c2] = staging[r*64 + c2]
                st_src = (
                    stage_t[:]
                    .rearrange("a (r c2) -> a r c2", r=16, c2=64)
                    .unsqueeze(1)
                    .broadcast_to([1, 8, 16, 64])
                    .bitcast(i16)
                    .squeeze(0)
                )
                nc.sync.dma_start(wg[:, mt * 64 : (mt + 1) * 64], st_src)

            def emit_gathers(b, mt):
                # two 512-idx gathers per tile (1024-descriptor SWDGE carveout
                # limit per instruction), round-robin over the 4 SWDGE queues.
                # idx stream position i = ((u*8 + k)*16 + r) within the tile,
                # i.e. token-half-major; conv rhs un-permutes with a 4D view.
                st = state[b]
                if mt % GROUP_TILES == 0 or mt == groups[-1][0]:
                    g = [gi for gi, (gs, gn) in enumerate(groups) if gs == mt][0]
                    gg = g_pool.tile(
                        [P, 1, GROUP_TILES * 1024], fp16, tag="g"
                    )
                    st["g"][g] = gg
                g = [gi for gi, (gs, gn) in enumerate(groups)
                     if gs <= mt < gs + gn][0]
                tloc = mt - groups[g][0]
                gg = st["g"][g]
                for half in range(2):
                    o = tloc * 1024 + half * 512
                    nc.gpsimd.dma_gather(
                        gg[:, :, o : o + 512],
                        st["tbl"][:],
                        st["widx"][:, mt * 64 + half * 32 : mt * 64 + half * 32 + 32],
                        512,
                        512,
                        D,
                        transpose=True,
                        queue_num=(2 * mt + half) % 4,
                        sbuf_tokens_per_rank=P,
                        sbuf_free_dim_per_rank=2 * D,
                    )

            def emit_conv(b, g):
                st = state[b]
                xhi = st["xhi"]
                gstart, gtiles = groups[g]
                gtok = gtiles * P
                g0 = gstart * P
                gv = st["g"].pop(g)[:, 0, : gtiles * 1024].rearrange(
                    "d (t u k r) -> d t u k r", t=gtiles, u=8, k=8, r=16
                )
                for h in range(2):
                    cp = psum_conv_pool.tile([P, CHUNK], f32, tag="pconv")
                    # k = 0: self columns from the fp16 x2 copy
                    nc.tensor.matmul(
                        cp[:, :gtok],
                        lhsT=wt_v[:, 0, h, :],
                        rhs=xhi[:, g0 : g0 + gtok],
                        start=True, stop=False,
                    )
                    for k in range(1, K):
                        nc.tensor.matmul(
                            cp[:, :gtok],
                            lhsT=wt_v[:, k, h, :],
                            rhs=gv[:, :, :, k - 1, :],
                            start=False, stop=(k == K - 1),
                        )
                    o_s = outs_pool.tile([P, CHUNK], f32, tag="outs")
                    nc.scalar.activation(
                        o_s[:, :gtok], cp[:, :gtok],
                        mybir.ActivationFunctionType.Relu,
                        bias=bias_s[:, h : h + 1],
                    )
                    nc.scalar.dma_start(
                        out_d.ap()[b, h * P : (h + 1) * P, g0 : g0 + gtok],
                        o_s[:, :gtok],
                    )
                if g == len(groups) - 1:
                    state.pop(b, None)

            # ---- software-pipelined emission over flat tile slots ----
            group_end = {gs + gn - 1: gi for gi, (gs, gn) in enumerate(groups)}

            emit_bstart(0)
            for s in range(T + CONV_DELAY + 1):
                if s + BSTART_LEAD < T and (s + BSTART_LEAD) % nt == 0:
                    emit_bstart((s + BSTART_LEAD) // nt)
                if s < T:
                    b, mt = divmod(s, nt)
                    emit_scores(b, mt)
                    emit_topk(b, mt)
                    emit_shuffle(b, mt)
                t_g = s - GATHER_LAG
                if 0 <= t_g < T:
                    b, mt = divmod(t_g, nt)
                    emit_gathers(b, mt)
                t_c = s - CONV_DELAY
                if 0 <= t_c < T:
                    b, mt = divmod(t_c, nt)
                    if mt in group_end:
                        emit_conv(b, group_end[mt])

    nc.compile()
    _cache[key] = nc
    return nc, None


def _host_inputs(x, conv_w, conv_b):
    """Per-batch device inputs + shared per-core constants."""
    x = np.ascontiguousarray(x, dtype=np.float32)
    b = x.shape[0]
    x1 = (
        x.reshape(b, C_IN, H // S, S, W // S, S)
        .transpose(0, 1, 3, 5, 2, 4)
        .reshape(b, D, N)
    )
    xhi = x1.astype(np.float16)
    # SBUF gather table: token t = r*128 + p lives at [part p, r*128 : (r+1)*128]
    x2t = np.ascontiguousarray(
        xhi.transpose(0, 2, 1)
        .reshape(b, N // 128, 128, D)
        .transpose(0, 2, 1, 3)
        .reshape(b, 128, N // 128 * D)
    )
    v = -0.5 * np.einsum("bdn,bdn->bn", x1.astype(np.float64), x1.astype(np.float64))
    nhi = v.astype(np.float16)
    nlo = (v - nhi.astype(np.float64)).astype(np.float16)
    nsq2 = np.stack([nhi, nlo], axis=1)                           # [B, 2, N]
    per_batch = dict(x2=x1, x2t=x2t, xhi=np.ascontiguousarray(xhi), nsq2=nsq2)

    wt = np.ascontiguousarray(
        conv_w.reshape(2, P, D, K).transpose(2, 3, 0, 1), dtype=np.float16
    )  # [D, K, 2, P]
    bias = np.ascontiguousarray(
        conv_b.reshape(2, P).transpose(1, 0), dtype=np.float32
    )  # [P, 2]
    ones2 = np.ones((2, P), dtype=np.float16)
    negbig = (NEGBIG * np.eye(P)).astype(np.float16)
    ident = np.eye(P, dtype=np.float16)
    consts = dict(wt=wt, bias=bias, ones2=ones2, negbig=negbig, ident=ident)
    return per_batch, consts


def kernel(x, conv_w, conv_b):
    nc, _ = _build_kernel()
    per_batch, consts = _host_inputs(x, conv_w, conv_b)
    in_maps = []
    for c in range(NCORES):
        m = dict(consts)
        for k, v in per_batch.items():
            m[k] = np.ascontiguousarray(v[c * BPC : (c + 1) * BPC])
        in_maps.append(m)
    res = bass_utils.run_bass_kernel_spmd(nc, in_maps, core_ids=list(range(NCORES)))
    outs = np.concatenate([r["out"] for r in res.results], axis=0)  # [B, 256, N]
    # pixel shuffle back: channel dim = (co, sy, sx); token = (h, w)
    o = outs.reshape(B, C_OUT, S, S, H // S, W // S)
    o = o.transpose(0, 1, 4, 2, 5, 3).reshape(B, C_OUT, H, W)
    return np.ascontiguousarray(o, dtype=np.float32)


# revision 24
# speedup vs baseline: 1.0961x; 1.0025x over previous
"""Trainium2 Bass kernel for Conv2d_NN (k-NN gather + grouped conv1d).

Shapes (hardcoded): x (32, 32, 96, 96) f32, conv_w (256, 128, 9) f32,
conv_b (256,) f32 -> out (32, 64, 96, 96) f32.

Strategy: data-parallel over batch across 8 NeuronCores (4 batches/core).
Per batch on device (tokens N=2304, features D=128 after host pixel-unshuffle):
  - scores = x2^T @ x2 - 0.5*||x_j||^2 in fp32 on PE (fp32 matmul streams at
    ~2 cyc/col on HW), PSUM-chunked [128,512]x5; the j-dependent nsq term is
    one 2-row fp16 matmul per chunk (host-prepared exact hi+lo split); self
    excluded with a -16384 fp16 diag matmul.  Ranking needs fp32-exact
    scores: fp16/bf16 storage or single-fp16-product dots flip neighbors
    and push rel err to 5e-2..1e-1 (measured on host).
  - ACT evacuates PSUM chunks to an fp32 scores row-block; DVE max8 /
    find_index8 give the top-8 neighbor indices per token.
  - a 2-hop DMA shuffle (SBUF->DRAM->SBUF broadcast) rewraps the [128,8]
    index tile into the 16-partition-wrapped layout, k-major per 512-token
    conv group.
  - dma_gather (SWDGE, transpose mode) pulls neighbor token rows from a
    host-prepared fp16 [N,128] DRAM table straight into conv-rhs layout
    [128 feat, 8*512 tok].  This replaces the baseline's gpsimd ap_gather,
    which ran ~250us per call on the DSP cores and serialized the whole
    kernel (2.15ms); descriptor-generated DMA does the same gather in ~10us
    and overlaps with compute.
  - conv1d = 9 accumulating 128x128 fp16 matmuls per output half (k=0 rhs is
    the fp16 x2 copy, k=1..8 slices of the gathered buffer); ACT adds bias +
    ReLU; DMA writes (b, 256, N) fp32.  fp16 conv adds ~3e-4 rel err.
Host does pixel-unshuffle/shuffle and all dtype prep (fp16 table, nsq
hi/lo rows, fp16 weights).
"""

import sys

for _p in ("/opt/trn_rl_repo",):
    if _p not in sys.path:
        sys.path.insert(0, _p)

import numpy as np

import concourse.bass as bass
import concourse.mybir as mybir
import concourse.tile as tile
from concourse import bacc, bass_utils

# Problem constants
B, C_IN, C_OUT, H, W = 32, 32, 64, 96, 96
S = 2
K = 9
D = C_IN * S * S            # 128
D_OUT = C_OUT * S * S       # 256
N = (H // S) * (W // S)     # 2304
NCORES = 8
BPC = B // NCORES           # 4 batches per core

P = 128                     # partitions / m-tile size
NT = N // P                 # 18 m-tiles
CHUNK = 512                 # psum bank = 512 f32; conv group chunk
SCHUNK = 512                # scores psum chunk = 1 bank (matmul cannot cross banks)
CHUNKS = [(c, min(SCHUNK, N - c)) for c in range(0, N, SCHUNK)]  # 4x512 + 256
NEGBIG = -16384.0           # fp16-exact, dominates any real score
GROUP_TILES = 4             # m-tiles per conv group (512 tokens)

# pipeline lags (in tile slots)
GATHER_LAG = 1              # gather emitted this many slots after its group ends
BSTART_LEAD = 6             # batch-start work emitted this many slots early
CONV_DELAY = 7              # conv emitted this many slots after its group ends

_cache = {}


def _build_kernel(bpc=BPC, nt=NT):
    key = ("nc", bpc, nt)
    if key in _cache:
        return _cache[key], None

    nc = bacc.Bacc(
        "TRN2", target_bir_lowering=False, debug=False, num_swdge_queues=4
    )

    f32 = mybir.dt.float32
    fp16 = mybir.dt.float16
    u16 = mybir.dt.uint16
    i16 = mybir.dt.int16

    n_tok = nt * P

    # groups per batch: (start_tile, n_tiles)
    groups = []
    mt = 0
    while mt < nt:
        gt = min(GROUP_TILES, nt - mt)
        groups.append((mt, gt))
        mt += gt
    widx_w = nt * 64                             # 64 wrapped cols per tile

    # I/O
    x2_d = nc.dram_tensor("x2", [bpc, D, n_tok], f32, kind="ExternalInput")
    x2t_d = nc.dram_tensor("x2t", [bpc, P, (n_tok // P) * P], fp16, kind="ExternalInput")
    xhi_d = nc.dram_tensor("xhi", [bpc, D, n_tok], fp16, kind="ExternalInput")
    nsq2_d = nc.dram_tensor("nsq2", [bpc, 2, n_tok], fp16, kind="ExternalInput")
    wt_d = nc.dram_tensor("wt", [D, K, 2, P], fp16, kind="ExternalInput")
    bias_d = nc.dram_tensor("bias", [P, 2], f32, kind="ExternalInput")
    ones2_d = nc.dram_tensor("ones2", [2, P], fp16, kind="ExternalInput")
    negbig_d = nc.dram_tensor("negbig", [P, P], fp16, kind="ExternalInput")
    ident_d = nc.dram_tensor("ident", [P, P], fp16, kind="ExternalInput")
    out_d = nc.dram_tensor("out", [bpc, D_OUT, n_tok], f32, kind="ExternalOutput")

    T = bpc * nt                                 # total tile slots

    with tile.TileContext(nc) as tc:
        import contextlib

        with contextlib.ExitStack() as ctx:
            const_pool = ctx.enter_context(tc.tile_pool(name="consts", bufs=1))
            x2_pool = ctx.enter_context(tc.tile_pool(name="x2", bufs=2))
            xhi_pool = ctx.enter_context(tc.tile_pool(name="xhi", bufs=2))
            tbl_pool = ctx.enter_context(tc.tile_pool(name="tbl", bufs=2))
            nsq_pool = ctx.enter_context(tc.tile_pool(name="nsq", bufs=2))
            scores_pool = ctx.enter_context(tc.tile_pool(name="scores", bufs=4))
            mx_pool = ctx.enter_context(tc.tile_pool(name="mx", bufs=8))
            widx_pool = ctx.enter_context(tc.tile_pool(name="widx", bufs=2))
            g_pool = ctx.enter_context(tc.tile_pool(name="g", bufs=4))
            outs_pool = ctx.enter_context(tc.tile_pool(name="outs", bufs=3))
            psum_sc_pool = ctx.enter_context(
                tc.tile_pool(name="psums", bufs=5, space="PSUM")
            )
            psum_conv_pool = ctx.enter_context(
                tc.tile_pool(name="psumc", bufs=2, space="PSUM")
            )
            dram_pool = ctx.enter_context(
                tc.tile_pool(name="stage", bufs=8, space="DRAM")
            )

            # constants, loaded once
            wt_s = const_pool.tile([D, K * 2 * P], fp16, tag="wt")
            nc.sync.dma_start(wt_s[:], wt_d.ap().rearrange("d k h c -> d (k h c)"))
            wt_v = wt_s[:].rearrange("d (k h c) -> d k h c", k=K, h=2, c=P)
            bias_s = const_pool.tile([P, 2], f32, tag="bias")
            nc.sync.dma_start(bias_s[:], bias_d.ap())
            ones2_s = const_pool.tile([2, P], fp16, tag="ones2")
            nc.sync.dma_start(ones2_s[:], ones2_d.ap())
            negbig_s = const_pool.tile([P, P], fp16, tag="negbig")
            nc.sync.dma_start(negbig_s[:], negbig_d.ap())
            ident_s = const_pool.tile([P, P], fp16, tag="ident")
            nc.sync.dma_start(ident_s[:], ident_d.ap())

            # per-batch state (rotating pool tiles), keyed by batch
            state = {}

            def emit_bstart(b):
                x2 = x2_pool.tile([D, n_tok], f32, tag="x2")
                nc.scalar.dma_start(x2[:], x2_d.ap()[b])
                xhi = xhi_pool.tile([D, n_tok], fp16, tag="xhi")
                nc.scalar.dma_start(xhi[:], xhi_d.ap()[b])
                nsq2 = nsq_pool.tile([2, n_tok], fp16, tag="nsq2")
                nc.scalar.dma_start(nsq2[:], nsq2_d.ap()[b])
                tbl = tbl_pool.tile([P, n_tok], fp16, tag="tbl")
                nc.scalar.dma_start(tbl[:], x2t_d.ap()[b])
                widx = widx_pool.tile([P, widx_w], i16, tag="widx")
                state[b] = dict(
                    x2=x2, xhi=xhi, nsq2=nsq2, tbl=tbl, widx=widx, g={}
                )

            def emit_scores(b, mt):
                st = state[b]
                x2 = st["x2"]
                nsq2 = st["nsq2"]
                m0 = mt * P
                dc = m0 // SCHUNK
                pscs = []
                for ci, (c0, w) in enumerate(CHUNKS):
                    psc = psum_sc_pool.tile([P, SCHUNK], f32, tag="psc")
                    pscs.append(psc)
                    nc.tensor.matmul(
                        psc[:, :w],
                        lhsT=x2[:, m0 : m0 + P],
                        rhs=x2[:, c0 : c0 + w],
                        start=True, stop=False,
                    )
                # self-exclusion: scores[p, m0+p] += NEGBIG
                d0 = m0 - dc * SCHUNK
                nc.tensor.matmul(
                    pscs[dc][:, d0 : d0 + P],
                    lhsT=negbig_s[:],
                    rhs=ident_s[:],
                    start=False, stop=False,
                )
                # j-dependent -0.5*nsq[j] as one 2-row fp16 matmul per chunk
                for ci, (c0, w) in enumerate(CHUNKS):
                    nc.tensor.matmul(
                        pscs[ci][:, :w],
                        lhsT=ones2_s[:],
                        rhs=nsq2[:, c0 : c0 + w],
                        start=False, stop=True,
                    )
                scores = scores_pool.tile([P, n_tok], f32, tag="scores")
                for ci, (c0, w) in enumerate(CHUNKS):
                    nc.scalar.copy(scores[:, c0 : c0 + w], pscs[ci][:, :w])
                st["scores"] = st.get("scores", {})
                st["scores"][mt] = scores

            def emit_topk(b, mt):
                st = state[b]
                scores = st["scores"].pop(mt)
                mx8 = mx_pool.tile([P, 8], f32, tag="mx8")
                nc.vector.max(out=mx8[:], in_=scores[:])
                midx = mx_pool.tile([P, 8], u16, tag="midx")
                nc.vector.max_index(midx[:], mx8[:], scores[:])
                st["midx"] = st.get("midx", {})
                st["midx"][mt] = midx

            def emit_shuffle(b, mt):
                st = state[b]
                midx = st["midx"].pop(mt)
                wg = st["widx"]
                # hop 1: midx [128,8] -> staging[(r*8 + u)*8 + k] (DRAM);
                # k innermost keeps both hops at 16B-contiguous DMA runs
                stage_t = dram_pool.tile([1, 1024], u16, tag="stage")
                st_dst = stage_t[:].rearrange(
                    "a (r u k) -> a u r k", r=16, u=8, k=8
                ).squeeze(0)
                nc.sync.dma_start(st_dst, midx[:])
                # hop 2: widx[16c+r, mt*64 + c2] = staging[r*64 + c2]
                st_src = (
                    stage_t[:]
                    .rearrange("a (r c2) -> a r c2", r=16, c2=64)
                    .unsqueeze(1)
                    .broadcast_to([1, 8, 16, 64])
                    .bitcast(i16)
                    .squeeze(0)
                )
                nc.sync.dma_start(wg[:, mt * 64 : (mt + 1) * 64], st_src)

            def emit_gathers(b, mt):
                # two 512-idx gathers per tile (1024-descriptor SWDGE carveout
                # limit per instruction), round-robin over the 4 SWDGE queues.
                # idx stream position i = ((u*8 + k)*16 + r) within the tile,
                # i.e. token-half-major; conv rhs un-permutes with a 4D view.
                st = state[b]
                if mt % GROUP_TILES == 0 or mt == groups[-1][0]:
                    g = [gi for gi, (gs, gn) in enumerate(groups) if gs == mt][0]
                    gg = g_pool.tile(
                        [P, 1, GROUP_TILES * 1024], fp16, tag="g"
                    )
                    st["g"][g] = gg
                g = [gi for gi, (gs, gn) in enumerate(groups)
                     if gs <= mt < gs + gn][0]
                tloc = mt - groups[g][0]
                gg = st["g"][g]
                for half in range(2):
                    o = tloc * 1024 + half * 512
                    nc.gpsimd.dma_gather(
                        gg[:, :, o : o + 512],
                        st["tbl"][:],
                        st["widx"][:, mt * 64 + half * 32 : mt * 64 + half * 32 + 32],
                        512,
                        512,
                        D,
                        transpose=True,
                        queue_num=(2 * mt + half) % 4,
                        sbuf_tokens_per_rank=P,
                        sbuf_free_dim_per_rank=2 * D,
                    )

            def emit_conv(b, g):
                st = state[b]
                xhi = st["xhi"]
                gstart, gtiles = groups[g]
                gtok = gtiles * P
                g0 = gstart * P
                gv = st["g"].pop(g)[:, 0, : gtiles * 1024].rearrange(
                    "d (t u k r) -> d t u k r", t=gtiles, u=8, k=8, r=16
                )
                for h in range(2):
                    cp = psum_conv_pool.tile([P, CHUNK], f32, tag="pconv")
                    # k = 0: self columns from the fp16 x2 copy
                    nc.tensor.matmul(
                        cp[:, :gtok],
                        lhsT=wt_v[:, 0, h, :],
                        rhs=xhi[:, g0 : g0 + gtok],
                        start=True, stop=False,
                    )
                    for k in range(1, K):
                        nc.tensor.matmul(
                            cp[:, :gtok],
                            lhsT=wt_v[:, k, h, :],
                            rhs=gv[:, :, :, k - 1, :],
                            start=False, stop=(k == K - 1),
                        )
                    o_s = outs_pool.tile([P, CHUNK], f32, tag="outs")
                    nc.scalar.activation(
                        o_s[:, :gtok], cp[:, :gtok],
                        mybir.ActivationFunctionType.Relu,
                        bias=bias_s[:, h : h + 1],
                    )
                    nc.scalar.dma_start(
                        out_d.ap()[b, h * P : (h + 1) * P, g0 : g0 + gtok],
                        o_s[:, :gtok],
                    )
                if g == len(groups) - 1:
                    state.pop(b, None)

            # ---- software-pipelined emission over flat tile slots ----
            group_end = {gs + gn - 1: gi for gi, (gs, gn) in enumerate(groups)}

            emit_bstart(0)
            for s in range(T + CONV_DELAY + 1):
                if s + BSTART_LEAD < T and (s + BSTART_LEAD) % nt == 0:
                    emit_bstart((s + BSTART_LEAD) // nt)
                if s < T:
                    b, mt = divmod(s, nt)
                    emit_scores(b, mt)
                    emit_topk(b, mt)
                    emit_shuffle(b, mt)
                t_g = s - GATHER_LAG
                if 0 <= t_g < T:
                    b, mt = divmod(t_g, nt)
                    emit_gathers(b, mt)
                t_c = s - CONV_DELAY
                if 0 <= t_c < T:
                    b, mt = divmod(t_c, nt)
                    if mt in group_end:
                        emit_conv(b, group_end[mt])

    nc.compile()
    _cache[key] = nc
    return nc, None


def _host_inputs(x, conv_w, conv_b):
    """Per-batch device inputs + shared per-core constants."""
    x = np.ascontiguousarray(x, dtype=np.float32)
    b = x.shape[0]
    x1 = (
        x.reshape(b, C_IN, H // S, S, W // S, S)
        .transpose(0, 1, 3, 5, 2, 4)
        .reshape(b, D, N)
    )
    xhi = x1.astype(np.float16)
    # SBUF gather table: token t = r*128 + p lives at [part p, r*128 : (r+1)*128]
    x2t = np.ascontiguousarray(
        xhi.transpose(0, 2, 1)
        .reshape(b, N // 128, 128, D)
        .transpose(0, 2, 1, 3)
        .reshape(b, 128, N // 128 * D)
    )
    v = -0.5 * np.einsum("bdn,bdn->bn", x1.astype(np.float64), x1.astype(np.float64))
    nhi = v.astype(np.float16)
    nlo = (v - nhi.astype(np.float64)).astype(np.float16)
    nsq2 = np.stack([nhi, nlo], axis=1)                           # [B, 2, N]
    per_batch = dict(x2=x1, x2t=x2t, xhi=np.ascontiguousarray(xhi), nsq2=nsq2)

    wt = np.ascontiguousarray(
        conv_w.reshape(2, P, D, K).transpose(2, 3, 0, 1), dtype=np.float16
    )  # [D, K, 2, P]
    bias = np.ascontiguousarray(
        conv_b.reshape(2, P).transpose(1, 0), dtype=np.float32
    )  # [P, 2]
    ones2 = np.ones((2, P), dtype=np.float16)
    negbig = (NEGBIG * np.eye(P)).astype(np.float16)
    ident = np.eye(P, dtype=np.float16)
    consts = dict(wt=wt, bias=bias, ones2=ones2, negbig=negbig, ident=ident)
    return per_batch, consts


def kernel(x, conv_w, conv_b):
    nc, _ = _build_kernel()
    per_batch, consts = _host_inputs(x, conv_w, conv_b)
    in_maps = []
    for c in range(NCORES):
        m = dict(consts)
        for k, v in per_batch.items():
            m[k] = np.ascontiguousarray(v[c * BPC : (c + 1) * BPC])
        in_maps.append(m)
    res = bass_utils.run_bass_kernel_spmd(nc, in_maps, core_ids=list(range(NCORES)))
    outs = np.concatenate([r["out"] for r in res.results], axis=0)  # [B, 256, N]
    # pixel shuffle back: channel dim = (co, sy, sx); token = (h, w)
    o = outs.reshape(B, C_OUT, S, S, H // S, W // S)
    o = o.transpose(0, 1, 4, 2, 5, 3).reshape(B, C_OUT, H, W)
    return np.ascontiguousarray(o, dtype=np.float32)


# revision 25
# speedup vs baseline: 1.1348x; 1.0353x over previous
"""Trainium2 Bass kernel for Conv2d_NN (k-NN gather + grouped conv1d).

Shapes (hardcoded): x (32, 32, 96, 96) f32, conv_w (256, 128, 9) f32,
conv_b (256,) f32 -> out (32, 64, 96, 96) f32.

Strategy: data-parallel over batch across 8 NeuronCores (4 batches/core).
Per batch on device (tokens N=2304, features D=128 after host pixel-unshuffle):
  - scores = x2^T @ x2 - 0.5*||x_j||^2 in fp32 on PE (fp32 matmul streams at
    ~2 cyc/col on HW), PSUM-chunked [128,512]x5; the j-dependent nsq term is
    one 2-row fp16 matmul per chunk (host-prepared exact hi+lo split); self
    excluded with a -16384 fp16 diag matmul.  Ranking needs fp32-exact
    scores: fp16/bf16 storage or single-fp16-product dots flip neighbors
    and push rel err to 5e-2..1e-1 (measured on host).
  - ACT evacuates PSUM chunks to an fp32 scores row-block; DVE max8 /
    find_index8 give the top-8 neighbor indices per token.
  - a 2-hop DMA shuffle (SBUF->DRAM->SBUF broadcast) rewraps the [128,8]
    index tile into the 16-partition-wrapped layout, k-major per 512-token
    conv group.
  - dma_gather (SWDGE, transpose mode) pulls neighbor token rows from a
    host-prepared fp16 [N,128] DRAM table straight into conv-rhs layout
    [128 feat, 8*512 tok].  This replaces the baseline's gpsimd ap_gather,
    which ran ~250us per call on the DSP cores and serialized the whole
    kernel (2.15ms); descriptor-generated DMA does the same gather in ~10us
    and overlaps with compute.
  - conv1d = 9 accumulating 128x128 fp16 matmuls per output half (k=0 rhs is
    the fp16 x2 copy, k=1..8 slices of the gathered buffer); ACT adds bias +
    ReLU; DMA writes (b, 256, N) fp32.  fp16 conv adds ~3e-4 rel err.
Host does pixel-unshuffle/shuffle and all dtype prep (fp16 table, nsq
hi/lo rows, fp16 weights).
"""

import sys

for _p in ("/opt/trn_rl_repo",):
    if _p not in sys.path:
        sys.path.insert(0, _p)

import numpy as np

import concourse.bass as bass
import concourse.mybir as mybir
import concourse.tile as tile
from concourse import bacc, bass_utils

# Problem constants
B, C_IN, C_OUT, H, W = 32, 32, 64, 96, 96
S = 2
K = 9
D = C_IN * S * S            # 128
D_OUT = C_OUT * S * S       # 256
N = (H // S) * (W // S)     # 2304
NCORES = 8
BPC = B // NCORES           # 4 batches per core

P = 128                     # partitions / m-tile size
NT = N // P                 # 18 m-tiles
CHUNK = 512                 # psum bank = 512 f32; conv group chunk
SCHUNK = 512                # scores psum chunk = 1 bank (matmul cannot cross banks)
CHUNKS = [(c, min(SCHUNK, N - c)) for c in range(0, N, SCHUNK)]  # 4x512 + 256
NEGBIG = -16384.0           # fp16-exact, dominates any real score
GROUP_TILES = 4             # m-tiles per conv group (512 tokens)

# pipeline lags (in tile slots)
GATHER_LAG = 1              # gather emitted this many slots after its group ends
BSTART_LEAD = 6             # batch-start work emitted this many slots early
CONV_DELAY = 8              # conv emitted this many slots after its group ends

_cache = {}


def _build_kernel(bpc=BPC, nt=NT):
    key = ("nc", bpc, nt)
    if key in _cache:
        return _cache[key], None

    nc = bacc.Bacc(
        "TRN2", target_bir_lowering=False, debug=False, num_swdge_queues=4
    )

    f32 = mybir.dt.float32
    fp16 = mybir.dt.float16
    u16 = mybir.dt.uint16
    i16 = mybir.dt.int16

    n_tok = nt * P

    # groups per batch: (start_tile, n_tiles)
    groups = []
    mt = 0
    while mt < nt:
        gt = min(GROUP_TILES, nt - mt)
        groups.append((mt, gt))
        mt += gt
    widx_w = nt * 64                             # 64 wrapped cols per tile

    # I/O
    x2_d = nc.dram_tensor("x2", [bpc, D, n_tok], f32, kind="ExternalInput")
    x2t_d = nc.dram_tensor("x2t", [bpc, P, (n_tok // P) * P], fp16, kind="ExternalInput")
    xhi_d = nc.dram_tensor("xhi", [bpc, D, n_tok], fp16, kind="ExternalInput")
    nsq2_d = nc.dram_tensor("nsq2", [bpc, 2, n_tok], fp16, kind="ExternalInput")
    wt_d = nc.dram_tensor("wt", [D, K, 2, P], fp16, kind="ExternalInput")
    bias_d = nc.dram_tensor("bias", [P, 2], f32, kind="ExternalInput")
    ones2_d = nc.dram_tensor("ones2", [2, P], fp16, kind="ExternalInput")
    negbig_d = nc.dram_tensor("negbig", [P, P], fp16, kind="ExternalInput")
    ident_d = nc.dram_tensor("ident", [P, P], fp16, kind="ExternalInput")
    out_d = nc.dram_tensor("out", [bpc, D_OUT, n_tok], f32, kind="ExternalOutput")

    T = bpc * nt                                 # total tile slots

    with tile.TileContext(nc) as tc:
        import contextlib

        with contextlib.ExitStack() as ctx:
            const_pool = ctx.enter_context(tc.tile_pool(name="consts", bufs=1))
            x2_pool = ctx.enter_context(tc.tile_pool(name="x2", bufs=2))
            xhi_pool = ctx.enter_context(tc.tile_pool(name="xhi", bufs=2))
            tbl_pool = ctx.enter_context(tc.tile_pool(name="tbl", bufs=2))
            nsq_pool = ctx.enter_context(tc.tile_pool(name="nsq", bufs=2))
            scores_pool = ctx.enter_context(tc.tile_pool(name="scores", bufs=4))
            mx_pool = ctx.enter_context(tc.tile_pool(name="mx", bufs=8))
            widx_pool = ctx.enter_context(tc.tile_pool(name="widx", bufs=2))
            g_pool = ctx.enter_context(tc.tile_pool(name="g", bufs=4))
            outs_pool = ctx.enter_context(tc.tile_pool(name="outs", bufs=3))
            psum_sc_pool = ctx.enter_context(
                tc.tile_pool(name="psums", bufs=5, space="PSUM")
            )
            psum_conv_pool = ctx.enter_context(
                tc.tile_pool(name="psumc", bufs=2, space="PSUM")
            )
            dram_pool = ctx.enter_context(
                tc.tile_pool(name="stage", bufs=8, space="DRAM")
            )

            # constants, loaded once
            wt_s = const_pool.tile([D, K * 2 * P], fp16, tag="wt")
            nc.sync.dma_start(wt_s[:], wt_d.ap().rearrange("d k h c -> d (k h c)"))
            wt_v = wt_s[:].rearrange("d (k h c) -> d k h c", k=K, h=2, c=P)
            bias_s = const_pool.tile([P, 2], f32, tag="bias")
            nc.sync.dma_start(bias_s[:], bias_d.ap())
            ones2_s = const_pool.tile([2, P], fp16, tag="ones2")
            nc.sync.dma_start(ones2_s[:], ones2_d.ap())
            negbig_s = const_pool.tile([P, P], fp16, tag="negbig")
            nc.sync.dma_start(negbig_s[:], negbig_d.ap())
            ident_s = const_pool.tile([P, P], fp16, tag="ident")
            nc.sync.dma_start(ident_s[:], ident_d.ap())

            # per-batch state (rotating pool tiles), keyed by batch
            state = {}

            def emit_bstart(b):
                x2 = x2_pool.tile([D, n_tok], f32, tag="x2")
                nc.scalar.dma_start(x2[:], x2_d.ap()[b])
                xhi = xhi_pool.tile([D, n_tok], fp16, tag="xhi")
                nc.scalar.dma_start(xhi[:], xhi_d.ap()[b])
                nsq2 = nsq_pool.tile([2, n_tok], fp16, tag="nsq2")
                nc.scalar.dma_start(nsq2[:], nsq2_d.ap()[b])
                tbl = tbl_pool.tile([P, n_tok], fp16, tag="tbl")
                nc.scalar.dma_start(tbl[:], x2t_d.ap()[b])
                widx = widx_pool.tile([P, widx_w], i16, tag="widx")
                state[b] = dict(
                    x2=x2, xhi=xhi, nsq2=nsq2, tbl=tbl, widx=widx, g={}
                )

            def emit_scores(b, mt):
                st = state[b]
                x2 = st["x2"]
                nsq2 = st["nsq2"]
                m0 = mt * P
                dc = m0 // SCHUNK
                pscs = []
                for ci, (c0, w) in enumerate(CHUNKS):
                    psc = psum_sc_pool.tile([P, SCHUNK], f32, tag="psc")
                    pscs.append(psc)
                    nc.tensor.matmul(
                        psc[:, :w],
                        lhsT=x2[:, m0 : m0 + P],
                        rhs=x2[:, c0 : c0 + w],
                        start=True, stop=False,
                    )
                # self-exclusion: scores[p, m0+p] += NEGBIG
                d0 = m0 - dc * SCHUNK
                nc.tensor.matmul(
                    pscs[dc][:, d0 : d0 + P],
                    lhsT=negbig_s[:],
                    rhs=ident_s[:],
                    start=False, stop=False,
                )
                # j-dependent -0.5*nsq[j] as one 2-row fp16 matmul per chunk
                for ci, (c0, w) in enumerate(CHUNKS):
                    nc.tensor.matmul(
                        pscs[ci][:, :w],
                        lhsT=ones2_s[:],
                        rhs=nsq2[:, c0 : c0 + w],
                        start=False, stop=True,
                    )
                scores = scores_pool.tile([P, n_tok], f32, tag="scores")
                for ci, (c0, w) in enumerate(CHUNKS):
                    nc.scalar.copy(scores[:, c0 : c0 + w], pscs[ci][:, :w])
                st["scores"] = st.get("scores", {})
                st["scores"][mt] = scores

            def emit_topk(b, mt):
                st = state[b]
                scores = st["scores"].pop(mt)
                mx8 = mx_pool.tile([P, 8], f32, tag="mx8")
                nc.vector.max(out=mx8[:], in_=scores[:])
                midx = mx_pool.tile([P, 8], u16, tag="midx")
                nc.vector.max_index(midx[:], mx8[:], scores[:])
                st["midx"] = st.get("midx", {})
                st["midx"][mt] = midx

            def emit_shuffle(b, mt):
                st = state[b]
                midx = st["midx"].pop(mt)
                wg = st["widx"]
                # hop 1: midx [128,8] -> staging[(r*8 + u)*8 + k] (DRAM);
                # k innermost keeps both hops at 16B-contiguous DMA runs
                stage_t = dram_pool.tile([1, 1024], u16, tag="stage")
                st_dst = stage_t[:].rearrange(
                    "a (r u k) -> a u r k", r=16, u=8, k=8
                ).squeeze(0)
                nc.sync.dma_start(st_dst, midx[:])
                # hop 2: widx[16c+r, mt*64 + c2] = staging[r*64 + c2]
                st_src = (
                    stage_t[:]
                    .rearrange("a (r c2) -> a r c2", r=16, c2=64)
                    .unsqueeze(1)
                    .broadcast_to([1, 8, 16, 64])
                    .bitcast(i16)
                    .squeeze(0)
                )
                nc.sync.dma_start(wg[:, mt * 64 : (mt + 1) * 64], st_src)

            def emit_gathers(b, mt):
                # two 512-idx gathers per tile (1024-descriptor SWDGE carveout
                # limit per instruction), round-robin over the 4 SWDGE queues.
                # idx stream position i = ((u*8 + k)*16 + r) within the tile,
                # i.e. token-half-major; conv rhs un-permutes with a 4D view.
                st = state[b]
                if mt % GROUP_TILES == 0 or mt == groups[-1][0]:
                    g = [gi for gi, (gs, gn) in enumerate(groups) if gs == mt][0]
                    gg = g_pool.tile(
                        [P, 1, GROUP_TILES * 1024], fp16, tag="g"
                    )
                    st["g"][g] = gg
                g = [gi for gi, (gs, gn) in enumerate(groups)
                     if gs <= mt < gs + gn][0]
                tloc = mt - groups[g][0]
                gg = st["g"][g]
                for half in range(2):
                    o = tloc * 1024 + half * 512
                    nc.gpsimd.dma_gather(
                        gg[:, :, o : o + 512],
                        st["tbl"][:],
                        st["widx"][:, mt * 64 + half * 32 : mt * 64 + half * 32 + 32],
                        512,
                        512,
                        D,
                        transpose=True,
                        queue_num=(2 * mt + half) % 4,
                        sbuf_tokens_per_rank=P,
                        sbuf_free_dim_per_rank=2 * D,
                    )

            def emit_conv(b, g):
                st = state[b]
                xhi = st["xhi"]
                gstart, gtiles = groups[g]
                gtok = gtiles * P
                g0 = gstart * P
                gv = st["g"].pop(g)[:, 0, : gtiles * 1024].rearrange(
                    "d (t u k r) -> d t u k r", t=gtiles, u=8, k=8, r=16
                )
                for h in range(2):
                    cp = psum_conv_pool.tile([P, CHUNK], f32, tag="pconv")
                    # k = 0: self columns from the fp16 x2 copy
                    nc.tensor.matmul(
                        cp[:, :gtok],
                        lhsT=wt_v[:, 0, h, :],
                        rhs=xhi[:, g0 : g0 + gtok],
                        start=True, stop=False,
                    )
                    for k in range(1, K):
                        nc.tensor.matmul(
                            cp[:, :gtok],
                            lhsT=wt_v[:, k, h, :],
                            rhs=gv[:, :, :, k - 1, :],
                            start=False, stop=(k == K - 1),
                        )
                    o_s = outs_pool.tile([P, CHUNK], f32, tag="outs")
                    nc.scalar.activation(
                        o_s[:, :gtok], cp[:, :gtok],
                        mybir.ActivationFunctionType.Relu,
                        bias=bias_s[:, h : h + 1],
                    )
                    nc.scalar.dma_start(
                        out_d.ap()[b, h * P : (h + 1) * P, g0 : g0 + gtok],
                        o_s[:, :gtok],
                    )
                if g == len(groups) - 1:
                    state.pop(b, None)

            # ---- software-pipelined emission over flat tile slots ----
            group_end = {gs + gn - 1: gi for gi, (gs, gn) in enumerate(groups)}

            emit_bstart(0)
            for s in range(T + CONV_DELAY + 1):
                if s + BSTART_LEAD < T and (s + BSTART_LEAD) % nt == 0:
                    emit_bstart((s + BSTART_LEAD) // nt)
                if s < T:
                    b, mt = divmod(s, nt)
                    emit_scores(b, mt)
                    emit_topk(b, mt)
                    emit_shuffle(b, mt)
                t_g = s - GATHER_LAG
                if 0 <= t_g < T:
                    b, mt = divmod(t_g, nt)
                    emit_gathers(b, mt)
                t_c = s - CONV_DELAY
                if 0 <= t_c < T:
                    b, mt = divmod(t_c, nt)
                    if mt in group_end:
                        emit_conv(b, group_end[mt])

    nc.compile()
    _cache[key] = nc
    return nc, None


def _host_inputs(x, conv_w, conv_b):
    """Per-batch device inputs + shared per-core constants."""
    x = np.ascontiguousarray(x, dtype=np.float32)
    b = x.shape[0]
    x1 = (
        x.reshape(b, C_IN, H // S, S, W // S, S)
        .transpose(0, 1, 3, 5, 2, 4)
        .reshape(b, D, N)
    )
    xhi = x1.astype(np.float16)
    # SBUF gather table: token t = r*128 + p lives at [part p, r*128 : (r+1)*128]
    x2t = np.ascontiguousarray(
        xhi.transpose(0, 2, 1)
        .reshape(b, N // 128, 128, D)
        .transpose(0, 2, 1, 3)
        .reshape(b, 128, N // 128 * D)
    )
    v = -0.5 * np.einsum("bdn,bdn->bn", x1.astype(np.float64), x1.astype(np.float64))
    nhi = v.astype(np.float16)
    nlo = (v - nhi.astype(np.float64)).astype(np.float16)
    nsq2 = np.stack([nhi, nlo], axis=1)                           # [B, 2, N]
    per_batch = dict(x2=x1, x2t=x2t, xhi=np.ascontiguousarray(xhi), nsq2=nsq2)

    wt = np.ascontiguousarray(
        conv_w.reshape(2, P, D, K).transpose(2, 3, 0, 1), dtype=np.float16
    )  # [D, K, 2, P]
    bias = np.ascontiguousarray(
        conv_b.reshape(2, P).transpose(1, 0), dtype=np.float32
    )  # [P, 2]
    ones2 = np.ones((2, P), dtype=np.float16)
    negbig = (NEGBIG * np.eye(P)).astype(np.float16)
    ident = np.eye(P, dtype=np.float16)
    consts = dict(wt=wt, bias=bias, ones2=ones2, negbig=negbig, ident=ident)
    return per_batch, consts


def kernel(x, conv_w, conv_b):
    nc, _ = _build_kernel()
    per_batch, consts = _host_inputs(x, conv_w, conv_b)
    in_maps = []
    for c in range(NCORES):
        m = dict(consts)
        for k, v in per_batch.items():
            m[k] = np.ascontiguousarray(v[c * BPC : (c + 1) * BPC])
        in_maps.append(m)
    res = bass_utils.run_bass_kernel_spmd(nc, in_maps, core_ids=list(range(NCORES)))
    outs = np.concatenate([r["out"] for r in res.results], axis=0)  # [B, 256, N]
    # pixel shuffle back: channel dim = (co, sy, sx); token = (h, w)
    o = outs.reshape(B, C_OUT, S, S, H // S, W // S)
    o = o.transpose(0, 1, 4, 2, 5, 3).reshape(B, C_OUT, H, W)
    return np.ascontiguousarray(o, dtype=np.float32)


# revision 26
# speedup vs baseline: 1.1426x; 1.0069x over previous
"""Trainium2 Bass kernel for Conv2d_NN (k-NN gather + grouped conv1d).

Shapes (hardcoded): x (32, 32, 96, 96) f32, conv_w (256, 128, 9) f32,
conv_b (256,) f32 -> out (32, 64, 96, 96) f32.

Strategy: data-parallel over batch across 8 NeuronCores (4 batches/core).
Per batch on device (tokens N=2304, features D=128 after host pixel-unshuffle):
  - scores = x2^T @ x2 - 0.5*||x_j||^2 in fp32 on PE (fp32 matmul streams at
    ~2 cyc/col on HW), PSUM-chunked [128,512]x5; the j-dependent nsq term is
    one 2-row fp16 matmul per chunk (host-prepared exact hi+lo split); self
    excluded with a -16384 fp16 diag matmul.  Ranking needs fp32-exact
    scores: fp16/bf16 storage or single-fp16-product dots flip neighbors
    and push rel err to 5e-2..1e-1 (measured on host).
  - ACT evacuates PSUM chunks to an fp32 scores row-block; DVE max8 /
    find_index8 give the top-8 neighbor indices per token.
  - a 2-hop DMA shuffle (SBUF->DRAM->SBUF broadcast) rewraps the [128,8]
    index tile into the 16-partition-wrapped layout, k-major per 512-token
    conv group.
  - dma_gather (SWDGE, transpose mode) pulls neighbor token rows from a
    host-prepared fp16 [N,128] DRAM table straight into conv-rhs layout
    [128 feat, 8*512 tok].  This replaces the baseline's gpsimd ap_gather,
    which ran ~250us per call on the DSP cores and serialized the whole
    kernel (2.15ms); descriptor-generated DMA does the same gather in ~10us
    and overlaps with compute.
  - conv1d = 9 accumulating 128x128 fp16 matmuls per output half (k=0 rhs is
    the fp16 x2 copy, k=1..8 slices of the gathered buffer); ACT adds bias +
    ReLU; DMA writes (b, 256, N) fp32.  fp16 conv adds ~3e-4 rel err.
Host does pixel-unshuffle/shuffle and all dtype prep (fp16 table, nsq
hi/lo rows, fp16 weights).
"""

import sys

for _p in ("/opt/trn_rl_repo",):
    if _p not in sys.path:
        sys.path.insert(0, _p)

import numpy as np

import concourse.bass as bass
import concourse.mybir as mybir
import concourse.tile as tile
from concourse import bacc, bass_utils

# Problem constants
B, C_IN, C_OUT, H, W = 32, 32, 64, 96, 96
S = 2
K = 9
D = C_IN * S * S            # 128
D_OUT = C_OUT * S * S       # 256
N = (H // S) * (W // S)     # 2304
NCORES = 8
BPC = B // NCORES           # 4 batches per core

P = 128                     # partitions / m-tile size
NT = N // P                 # 18 m-tiles
CHUNK = 512                 # psum bank = 512 f32; conv group chunk
SCHUNK = 512                # scores psum chunk = 1 bank (matmul cannot cross banks)
CHUNKS = [(c, min(SCHUNK, N - c)) for c in range(0, N, SCHUNK)]  # 4x512 + 256
NEGBIG = -16384.0           # fp16-exact, dominates any real score
GROUP_TILES = 4             # m-tiles per conv group (512 tokens)

# pipeline lags (in tile slots)
GATHER_LAG = 1              # gather emitted this many slots after its group ends
BSTART_LEAD = 6             # batch-start work emitted this many slots early
CONV_DELAY = 10             # conv emitted this many slots after its group ends

_cache = {}


def _build_kernel(bpc=BPC, nt=NT):
    key = ("nc", bpc, nt)
    if key in _cache:
        return _cache[key], None

    nc = bacc.Bacc(
        "TRN2", target_bir_lowering=False, debug=False, num_swdge_queues=4
    )

    f32 = mybir.dt.float32
    fp16 = mybir.dt.float16
    u16 = mybir.dt.uint16
    i16 = mybir.dt.int16

    n_tok = nt * P

    # groups per batch: (start_tile, n_tiles)
    groups = []
    mt = 0
    while mt < nt:
        gt = min(GROUP_TILES, nt - mt)
        groups.append((mt, gt))
        mt += gt
    widx_w = nt * 64                             # 64 wrapped cols per tile

    # I/O
    x2_d = nc.dram_tensor("x2", [bpc, D, n_tok], f32, kind="ExternalInput")
    x2t_d = nc.dram_tensor("x2t", [bpc, P, (n_tok // P) * P], fp16, kind="ExternalInput")
    xhi_d = nc.dram_tensor("xhi", [bpc, D, n_tok], fp16, kind="ExternalInput")
    nsq2_d = nc.dram_tensor("nsq2", [bpc, 2, n_tok], fp16, kind="ExternalInput")
    wt_d = nc.dram_tensor("wt", [D, K, 2, P], fp16, kind="ExternalInput")
    bias_d = nc.dram_tensor("bias", [P, 2], f32, kind="ExternalInput")
    ones2_d = nc.dram_tensor("ones2", [2, P], fp16, kind="ExternalInput")
    negbig_d = nc.dram_tensor("negbig", [P, P], fp16, kind="ExternalInput")
    ident_d = nc.dram_tensor("ident", [P, P], fp16, kind="ExternalInput")
    out_d = nc.dram_tensor("out", [bpc, D_OUT, n_tok], f32, kind="ExternalOutput")

    T = bpc * nt                                 # total tile slots

    with tile.TileContext(nc) as tc:
        import contextlib

        with contextlib.ExitStack() as ctx:
            const_pool = ctx.enter_context(tc.tile_pool(name="consts", bufs=1))
            x2_pool = ctx.enter_context(tc.tile_pool(name="x2", bufs=2))
            xhi_pool = ctx.enter_context(tc.tile_pool(name="xhi", bufs=2))
            tbl_pool = ctx.enter_context(tc.tile_pool(name="tbl", bufs=2))
            nsq_pool = ctx.enter_context(tc.tile_pool(name="nsq", bufs=2))
            scores_pool = ctx.enter_context(tc.tile_pool(name="scores", bufs=4))
            mx_pool = ctx.enter_context(tc.tile_pool(name="mx", bufs=8))
            widx_pool = ctx.enter_context(tc.tile_pool(name="widx", bufs=2))
            g_pool = ctx.enter_context(tc.tile_pool(name="g", bufs=4))
            outs_pool = ctx.enter_context(tc.tile_pool(name="outs", bufs=3))
            psum_sc_pool = ctx.enter_context(
                tc.tile_pool(name="psums", bufs=5, space="PSUM")
            )
            psum_conv_pool = ctx.enter_context(
                tc.tile_pool(name="psumc", bufs=2, space="PSUM")
            )
            dram_pool = ctx.enter_context(
                tc.tile_pool(name="stage", bufs=8, space="DRAM")
            )

            # constants, loaded once
            wt_s = const_pool.tile([D, K * 2 * P], fp16, tag="wt")
            nc.sync.dma_start(wt_s[:], wt_d.ap().rearrange("d k h c -> d (k h c)"))
            wt_v = wt_s[:].rearrange("d (k h c) -> d k h c", k=K, h=2, c=P)
            bias_s = const_pool.tile([P, 2], f32, tag="bias")
            nc.sync.dma_start(bias_s[:], bias_d.ap())
            ones2_s = const_pool.tile([2, P], fp16, tag="ones2")
            nc.sync.dma_start(ones2_s[:], ones2_d.ap())
            negbig_s = const_pool.tile([P, P], fp16, tag="negbig")
            nc.sync.dma_start(negbig_s[:], negbig_d.ap())
            ident_s = const_pool.tile([P, P], fp16, tag="ident")
            nc.sync.dma_start(ident_s[:], ident_d.ap())

            # per-batch state (rotating pool tiles), keyed by batch
            state = {}

            def emit_bstart(b):
                x2 = x2_pool.tile([D, n_tok], f32, tag="x2")
                nc.scalar.dma_start(x2[:], x2_d.ap()[b])
                xhi = xhi_pool.tile([D, n_tok], fp16, tag="xhi")
                nc.scalar.dma_start(xhi[:], xhi_d.ap()[b])
                nsq2 = nsq_pool.tile([2, n_tok], fp16, tag="nsq2")
                nc.scalar.dma_start(nsq2[:], nsq2_d.ap()[b])
                tbl = tbl_pool.tile([P, n_tok], fp16, tag="tbl")
                nc.scalar.dma_start(tbl[:], x2t_d.ap()[b])
                widx = widx_pool.tile([P, widx_w], i16, tag="widx")
                state[b] = dict(
                    x2=x2, xhi=xhi, nsq2=nsq2, tbl=tbl, widx=widx, g={}
                )

            def emit_scores(b, mt):
                st = state[b]
                x2 = st["x2"]
                nsq2 = st["nsq2"]
                m0 = mt * P
                dc = m0 // SCHUNK
                pscs = []
                for ci, (c0, w) in enumerate(CHUNKS):
                    psc = psum_sc_pool.tile([P, SCHUNK], f32, tag="psc")
                    pscs.append(psc)
                    nc.tensor.matmul(
                        psc[:, :w],
                        lhsT=x2[:, m0 : m0 + P],
                        rhs=x2[:, c0 : c0 + w],
                        start=True, stop=False,
                    )
                # self-exclusion: scores[p, m0+p] += NEGBIG
                d0 = m0 - dc * SCHUNK
                nc.tensor.matmul(
                    pscs[dc][:, d0 : d0 + P],
                    lhsT=negbig_s[:],
                    rhs=ident_s[:],
                    start=False, stop=False,
                )
                # j-dependent -0.5*nsq[j] as one 2-row fp16 matmul per chunk
                for ci, (c0, w) in enumerate(CHUNKS):
                    nc.tensor.matmul(
                        pscs[ci][:, :w],
                        lhsT=ones2_s[:],
                        rhs=nsq2[:, c0 : c0 + w],
                        start=False, stop=True,
                    )
                scores = scores_pool.tile([P, n_tok], f32, tag="scores")
                for ci, (c0, w) in enumerate(CHUNKS):
                    nc.scalar.copy(scores[:, c0 : c0 + w], pscs[ci][:, :w])
                st["scores"] = st.get("scores", {})
                st["scores"][mt] = scores

            def emit_topk(b, mt):
                st = state[b]
                scores = st["scores"].pop(mt)
                mx8 = mx_pool.tile([P, 8], f32, tag="mx8")
                nc.vector.max(out=mx8[:], in_=scores[:])
                midx = mx_pool.tile([P, 8], u16, tag="midx")
                nc.vector.max_index(midx[:], mx8[:], scores[:])
                st["midx"] = st.get("midx", {})
                st["midx"][mt] = midx

            def emit_shuffle(b, mt):
                st = state[b]
                midx = st["midx"].pop(mt)
                wg = st["widx"]
                # hop 1: midx [128,8] -> staging[(r*8 + u)*8 + k] (DRAM);
                # k innermost keeps both hops at 16B-contiguous DMA runs
                stage_t = dram_pool.tile([1, 1024], u16, tag="stage")
                st_dst = stage_t[:].rearrange(
                    "a (r u k) -> a u r k", r=16, u=8, k=8
                ).squeeze(0)
                nc.sync.dma_start(st_dst, midx[:])
                # hop 2: widx[16c+r, mt*64 + c2] = staging[r*64 + c2]
                st_src = (
                    stage_t[:]
                    .rearrange("a (r c2) -> a r c2", r=16, c2=64)
                    .unsqueeze(1)
                    .broadcast_to([1, 8, 16, 64])
                    .bitcast(i16)
                    .squeeze(0)
                )
                nc.sync.dma_start(wg[:, mt * 64 : (mt + 1) * 64], st_src)

            def emit_gathers(b, mt):
                # two 512-idx gathers per tile (1024-descriptor SWDGE carveout
                # limit per instruction), round-robin over the 4 SWDGE queues.
                # idx stream position i = ((u*8 + k)*16 + r) within the tile,
                # i.e. token-half-major; conv rhs un-permutes with a 4D view.
                st = state[b]
                if mt % GROUP_TILES == 0 or mt == groups[-1][0]:
                    g = [gi for gi, (gs, gn) in enumerate(groups) if gs == mt][0]
                    gg = g_pool.tile(
                        [P, 1, GROUP_TILES * 1024], fp16, tag="g"
                    )
                    st["g"][g] = gg
                g = [gi for gi, (gs, gn) in enumerate(groups)
                     if gs <= mt < gs + gn][0]
                tloc = mt - groups[g][0]
                gg = st["g"][g]
                for half in range(2):
                    o = tloc * 1024 + half * 512
                    nc.gpsimd.dma_gather(
                        gg[:, :, o : o + 512],
                        st["tbl"][:],
                        st["widx"][:, mt * 64 + half * 32 : mt * 64 + half * 32 + 32],
                        512,
                        512,
                        D,
                        transpose=True,
                        queue_num=(2 * mt + half) % 4,
                        sbuf_tokens_per_rank=P,
                        sbuf_free_dim_per_rank=2 * D,
                    )

            def emit_conv(b, g):
                st = state[b]
                xhi = st["xhi"]
                gstart, gtiles = groups[g]
                gtok = gtiles * P
                g0 = gstart * P
                gv = st["g"].pop(g)[:, 0, : gtiles * 1024].rearrange(
                    "d (t u k r) -> d t u k r", t=gtiles, u=8, k=8, r=16
                )
                for h in range(2):
                    cp = psum_conv_pool.tile([P, CHUNK], f32, tag="pconv")
                    # k = 0: self columns from the fp16 x2 copy
                    nc.tensor.matmul(
                        cp[:, :gtok],
                        lhsT=wt_v[:, 0, h, :],
                        rhs=xhi[:, g0 : g0 + gtok],
                        start=True, stop=False,
                    )
                    for k in range(1, K):
                        nc.tensor.matmul(
                            cp[:, :gtok],
                            lhsT=wt_v[:, k, h, :],
                            rhs=gv[:, :, :, k - 1, :],
                            start=False, stop=(k == K - 1),
                        )
                    o_s = outs_pool.tile([P, CHUNK], f32, tag="outs")
                    nc.scalar.activation(
                        o_s[:, :gtok], cp[:, :gtok],
                        mybir.ActivationFunctionType.Relu,
                        bias=bias_s[:, h : h + 1],
                    )
                    nc.scalar.dma_start(
                        out_d.ap()[b, h * P : (h + 1) * P, g0 : g0 + gtok],
                        o_s[:, :gtok],
                    )
                if g == len(groups) - 1:
                    state.pop(b, None)

            # ---- software-pipelined emission over flat tile slots ----
            group_end = {gs + gn - 1: gi for gi, (gs, gn) in enumerate(groups)}

            emit_bstart(0)
            for s in range(T + CONV_DELAY + 1):
                if s + BSTART_LEAD < T and (s + BSTART_LEAD) % nt == 0:
                    emit_bstart((s + BSTART_LEAD) // nt)
                if s < T:
                    b, mt = divmod(s, nt)
                    emit_scores(b, mt)
                    emit_topk(b, mt)
                    emit_shuffle(b, mt)
                t_g = s - GATHER_LAG
                if 0 <= t_g < T:
                    b, mt = divmod(t_g, nt)
                    emit_gathers(b, mt)
                t_c = s - CONV_DELAY
                if 0 <= t_c < T:
                    b, mt = divmod(t_c, nt)
                    if mt in group_end:
                        emit_conv(b, group_end[mt])

    nc.compile()
    _cache[key] = nc
    return nc, None


def _host_inputs(x, conv_w, conv_b):
    """Per-batch device inputs + shared per-core constants."""
    x = np.ascontiguousarray(x, dtype=np.float32)
    b = x.shape[0]
    x1 = (
        x.reshape(b, C_IN, H // S, S, W // S, S)
        .transpose(0, 1, 3, 5, 2, 4)
        .reshape(b, D, N)
    )
    xhi = x1.astype(np.float16)
    # SBUF gather table: token t = r*128 + p lives at [part p, r*128 : (r+1)*128]
    x2t = np.ascontiguousarray(
        xhi.transpose(0, 2, 1)
        .reshape(b, N // 128, 128, D)
        .transpose(0, 2, 1, 3)
        .reshape(b, 128, N // 128 * D)
    )
    v = -0.5 * np.einsum("bdn,bdn->bn", x1.astype(np.float64), x1.astype(np.float64))
    nhi = v.astype(np.float16)
    nlo = (v - nhi.astype(np.float64)).astype(np.float16)
    nsq2 = np.stack([nhi, nlo], axis=1)                           # [B, 2, N]
    per_batch = dict(x2=x1, x2t=x2t, xhi=np.ascontiguousarray(xhi), nsq2=nsq2)

    wt = np.ascontiguousarray(
        conv_w.reshape(2, P, D, K).transpose(2, 3, 0, 1), dtype=np.float16
    )  # [D, K, 2, P]
    bias = np.ascontiguousarray(
        conv_b.reshape(2, P).transpose(1, 0), dtype=np.float32
    )  # [P, 2]
    ones2 = np.ones((2, P), dtype=np.float16)
    negbig = (NEGBIG * np.eye(P)).astype(np.float16)
    ident = np.eye(P, dtype=np.float16)
    consts = dict(wt=wt, bias=bias, ones2=ones2, negbig=negbig, ident=ident)
    return per_batch, consts


def kernel(x, conv_w, conv_b):
    nc, _ = _build_kernel()
    per_batch, consts = _host_inputs(x, conv_w, conv_b)
    in_maps = []
    for c in range(NCORES):
        m = dict(consts)
        for k, v in per_batch.items():
            m[k] = np.ascontiguousarray(v[c * BPC : (c + 1) * BPC])
        in_maps.append(m)
    res = bass_utils.run_bass_kernel_spmd(nc, in_maps, core_ids=list(range(NCORES)))
    outs = np.concatenate([r["out"] for r in res.results], axis=0)  # [B, 256, N]
    # pixel shuffle back: channel dim = (co, sy, sx); token = (h, w)
    o = outs.reshape(B, C_OUT, S, S, H // S, W // S)
    o = o.transpose(0, 1, 4, 2, 5, 3).reshape(B, C_OUT, H, W)
    return np.ascontiguousarray(o, dtype=np.float32)


# revision 27
# speedup vs baseline: 1.1428x; 1.0002x over previous
"""Trainium2 Bass kernel for Conv2d_NN (k-NN gather + grouped conv1d).

Shapes (hardcoded): x (32, 32, 96, 96) f32, conv_w (256, 128, 9) f32,
conv_b (256,) f32 -> out (32, 64, 96, 96) f32.

Strategy: data-parallel over batch across 8 NeuronCores (4 batches/core).
Per batch on device (tokens N=2304, features D=128 after host pixel-unshuffle):
  - scores = x2^T @ x2 - 0.5*||x_j||^2 in fp32 on PE (fp32 matmul streams at
    ~2 cyc/col on HW), PSUM-chunked [128,512]x5; the j-dependent nsq term is
    one 2-row fp16 matmul per chunk (host-prepared exact hi+lo split); self
    excluded with a -16384 fp16 diag matmul.  Ranking needs fp32-exact
    scores: fp16/bf16 storage or single-fp16-product dots flip neighbors
    and push rel err to 5e-2..1e-1 (measured on host).
  - ACT evacuates PSUM chunks to an fp32 scores row-block; DVE max8 /
    find_index8 give the top-8 neighbor indices per token.
  - a 2-hop DMA shuffle (SBUF->DRAM->SBUF broadcast) rewraps the [128,8]
    index tile into the 16-partition-wrapped layout, k-major per 512-token
    conv group.
  - dma_gather (SWDGE, transpose mode) pulls neighbor token rows from a
    host-prepared fp16 [N,128] DRAM table straight into conv-rhs layout
    [128 feat, 8*512 tok].  This replaces the baseline's gpsimd ap_gather,
    which ran ~250us per call on the DSP cores and serialized the whole
    kernel (2.15ms); descriptor-generated DMA does the same gather in ~10us
    and overlaps with compute.
  - conv1d = 9 accumulating 128x128 fp16 matmuls per output half (k=0 rhs is
    the fp16 x2 copy, k=1..8 slices of the gathered buffer); ACT adds bias +
    ReLU; DMA writes (b, 256, N) fp32.  fp16 conv adds ~3e-4 rel err.
Host does pixel-unshuffle/shuffle and all dtype prep (fp16 table, nsq
hi/lo rows, fp16 weights).
"""

import sys

for _p in ("/opt/trn_rl_repo",):
    if _p not in sys.path:
        sys.path.insert(0, _p)

import numpy as np

import concourse.bass as bass
import concourse.mybir as mybir
import concourse.tile as tile
from concourse import bacc, bass_utils

# Problem constants
B, C_IN, C_OUT, H, W = 32, 32, 64, 96, 96
S = 2
K = 9
D = C_IN * S * S            # 128
D_OUT = C_OUT * S * S       # 256
N = (H // S) * (W // S)     # 2304
NCORES = 8
BPC = B // NCORES           # 4 batches per core

P = 128                     # partitions / m-tile size
NT = N // P                 # 18 m-tiles
CHUNK = 512                 # psum bank = 512 f32; conv group chunk
SCHUNK = 512                # scores psum chunk = 1 bank (matmul cannot cross banks)
CHUNKS = [(c, min(SCHUNK, N - c)) for c in range(0, N, SCHUNK)]  # 4x512 + 256
NEGBIG = -16384.0           # fp16-exact, dominates any real score
GROUP_TILES = 4             # m-tiles per conv group (512 tokens)

# pipeline lags (in tile slots)
GATHER_LAG = 1              # gather emitted this many slots after its group ends
BSTART_LEAD = 6             # batch-start work emitted this many slots early
CONV_DELAY = 12             # conv emitted this many slots after its group ends

_cache = {}


def _build_kernel(bpc=BPC, nt=NT):
    key = ("nc", bpc, nt)
    if key in _cache:
        return _cache[key], None

    nc = bacc.Bacc(
        "TRN2", target_bir_lowering=False, debug=False, num_swdge_queues=4
    )

    f32 = mybir.dt.float32
    fp16 = mybir.dt.float16
    u16 = mybir.dt.uint16
    i16 = mybir.dt.int16

    n_tok = nt * P

    # groups per batch: (start_tile, n_tiles)
    groups = []
    mt = 0
    while mt < nt:
        gt = min(GROUP_TILES, nt - mt)
        groups.append((mt, gt))
        mt += gt
    widx_w = nt * 64                             # 64 wrapped cols per tile

    # I/O
    x2_d = nc.dram_tensor("x2", [bpc, D, n_tok], f32, kind="ExternalInput")
    x2t_d = nc.dram_tensor("x2t", [bpc, P, (n_tok // P) * P], fp16, kind="ExternalInput")
    xhi_d = nc.dram_tensor("xhi", [bpc, D, n_tok], fp16, kind="ExternalInput")
    nsq2_d = nc.dram_tensor("nsq2", [bpc, 2, n_tok], fp16, kind="ExternalInput")
    wt_d = nc.dram_tensor("wt", [D, K, 2, P], fp16, kind="ExternalInput")
    bias_d = nc.dram_tensor("bias", [P, 2], f32, kind="ExternalInput")
    ones2_d = nc.dram_tensor("ones2", [2, P], fp16, kind="ExternalInput")
    negbig_d = nc.dram_tensor("negbig", [P, P], fp16, kind="ExternalInput")
    ident_d = nc.dram_tensor("ident", [P, P], fp16, kind="ExternalInput")
    out_d = nc.dram_tensor("out", [bpc, D_OUT, n_tok], f32, kind="ExternalOutput")

    T = bpc * nt                                 # total tile slots

    with tile.TileContext(nc) as tc:
        import contextlib

        with contextlib.ExitStack() as ctx:
            const_pool = ctx.enter_context(tc.tile_pool(name="consts", bufs=1))
            x2_pool = ctx.enter_context(tc.tile_pool(name="x2", bufs=2))
            xhi_pool = ctx.enter_context(tc.tile_pool(name="xhi", bufs=2))
            tbl_pool = ctx.enter_context(tc.tile_pool(name="tbl", bufs=2))
            nsq_pool = ctx.enter_context(tc.tile_pool(name="nsq", bufs=2))
            scores_pool = ctx.enter_context(tc.tile_pool(name="scores", bufs=4))
            mx_pool = ctx.enter_context(tc.tile_pool(name="mx", bufs=8))
            widx_pool = ctx.enter_context(tc.tile_pool(name="widx", bufs=2))
            g_pool = ctx.enter_context(tc.tile_pool(name="g", bufs=5))
            outs_pool = ctx.enter_context(tc.tile_pool(name="outs", bufs=3))
            psum_sc_pool = ctx.enter_context(
                tc.tile_pool(name="psums", bufs=5, space="PSUM")
            )
            psum_conv_pool = ctx.enter_context(
                tc.tile_pool(name="psumc", bufs=2, space="PSUM")
            )
            dram_pool = ctx.enter_context(
                tc.tile_pool(name="stage", bufs=8, space="DRAM")
            )

            # constants, loaded once
            wt_s = const_pool.tile([D, K * 2 * P], fp16, tag="wt")
            nc.sync.dma_start(wt_s[:], wt_d.ap().rearrange("d k h c -> d (k h c)"))
            wt_v = wt_s[:].rearrange("d (k h c) -> d k h c", k=K, h=2, c=P)
            bias_s = const_pool.tile([P, 2], f32, tag="bias")
            nc.sync.dma_start(bias_s[:], bias_d.ap())
            ones2_s = const_pool.tile([2, P], fp16, tag="ones2")
            nc.sync.dma_start(ones2_s[:], ones2_d.ap())
            negbig_s = const_pool.tile([P, P], fp16, tag="negbig")
            nc.sync.dma_start(negbig_s[:], negbig_d.ap())
            ident_s = const_pool.tile([P, P], fp16, tag="ident")
            nc.sync.dma_start(ident_s[:], ident_d.ap())

            # per-batch state (rotating pool tiles), keyed by batch
            state = {}

            def emit_bstart(b):
                x2 = x2_pool.tile([D, n_tok], f32, tag="x2")
                nc.scalar.dma_start(x2[:], x2_d.ap()[b])
                xhi = xhi_pool.tile([D, n_tok], fp16, tag="xhi")
                nc.scalar.dma_start(xhi[:], xhi_d.ap()[b])
                nsq2 = nsq_pool.tile([2, n_tok], fp16, tag="nsq2")
                nc.scalar.dma_start(nsq2[:], nsq2_d.ap()[b])
                tbl = tbl_pool.tile([P, n_tok], fp16, tag="tbl")
                nc.scalar.dma_start(tbl[:], x2t_d.ap()[b])
                widx = widx_pool.tile([P, widx_w], i16, tag="widx")
                state[b] = dict(
                    x2=x2, xhi=xhi, nsq2=nsq2, tbl=tbl, widx=widx, g={}
                )

            def emit_scores(b, mt):
                st = state[b]
                x2 = st["x2"]
                nsq2 = st["nsq2"]
                m0 = mt * P
                dc = m0 // SCHUNK
                pscs = []
                for ci, (c0, w) in enumerate(CHUNKS):
                    psc = psum_sc_pool.tile([P, SCHUNK], f32, tag="psc")
                    pscs.append(psc)
                    nc.tensor.matmul(
                        psc[:, :w],
                        lhsT=x2[:, m0 : m0 + P],
                        rhs=x2[:, c0 : c0 + w],
                        start=True, stop=False,
                    )
                # self-exclusion: scores[p, m0+p] += NEGBIG
                d0 = m0 - dc * SCHUNK
                nc.tensor.matmul(
                    pscs[dc][:, d0 : d0 + P],
                    lhsT=negbig_s[:],
                    rhs=ident_s[:],
                    start=False, stop=False,
                )
                # j-dependent -0.5*nsq[j] as one 2-row fp16 matmul per chunk
                for ci, (c0, w) in enumerate(CHUNKS):
                    nc.tensor.matmul(
                        pscs[ci][:, :w],
                        lhsT=ones2_s[:],
                        rhs=nsq2[:, c0 : c0 + w],
                        start=False, stop=True,
                    )
                scores = scores_pool.tile([P, n_tok], f32, tag="scores")
                for ci, (c0, w) in enumerate(CHUNKS):
                    nc.scalar.copy(scores[:, c0 : c0 + w], pscs[ci][:, :w])
                st["scores"] = st.get("scores", {})
                st["scores"][mt] = scores

            def emit_topk(b, mt):
                st = state[b]
                scores = st["scores"].pop(mt)
                mx8 = mx_pool.tile([P, 8], f32, tag="mx8")
                nc.vector.max(out=mx8[:], in_=scores[:])
                midx = mx_pool.tile([P, 8], u16, tag="midx")
                nc.vector.max_index(midx[:], mx8[:], scores[:])
                st["midx"] = st.get("midx", {})
                st["midx"][mt] = midx

            def emit_shuffle(b, mt):
                st = state[b]
                midx = st["midx"].pop(mt)
                wg = st["widx"]
                # hop 1: midx [128,8] -> staging[(r*8 + u)*8 + k] (DRAM);
                # k innermost keeps both hops at 16B-contiguous DMA runs
                stage_t = dram_pool.tile([1, 1024], u16, tag="stage")
                st_dst = stage_t[:].rearrange(
                    "a (r u k) -> a u r k", r=16, u=8, k=8
                ).squeeze(0)
                nc.sync.dma_start(st_dst, midx[:])
                # hop 2: widx[16c+r, mt*64 + c2] = staging[r*64 + c2]
                st_src = (
                    stage_t[:]
                    .rearrange("a (r c2) -> a r c2", r=16, c2=64)
                    .unsqueeze(1)
                    .broadcast_to([1, 8, 16, 64])
                    .bitcast(i16)
                    .squeeze(0)
                )
                nc.sync.dma_start(wg[:, mt * 64 : (mt + 1) * 64], st_src)

            def emit_gathers(b, mt):
                # two 512-idx gathers per tile (1024-descriptor SWDGE carveout
                # limit per instruction), round-robin over the 4 SWDGE queues.
                # idx stream position i = ((u*8 + k)*16 + r) within the tile,
                # i.e. token-half-major; conv rhs un-permutes with a 4D view.
                st = state[b]
                if mt % GROUP_TILES == 0 or mt == groups[-1][0]:
                    g = [gi for gi, (gs, gn) in enumerate(groups) if gs == mt][0]
                    gg = g_pool.tile(
                        [P, 1, GROUP_TILES * 1024], fp16, tag="g"
                    )
                    st["g"][g] = gg
                g = [gi for gi, (gs, gn) in enumerate(groups)
                     if gs <= mt < gs + gn][0]
                tloc = mt - groups[g][0]
                gg = st["g"][g]
                for half in range(2):
                    o = tloc * 1024 + half * 512
                    nc.gpsimd.dma_gather(
                        gg[:, :, o : o + 512],
                        st["tbl"][:],
                        st["widx"][:, mt * 64 + half * 32 : mt * 64 + half * 32 + 32],
                        512,
                        512,
                        D,
                        transpose=True,
                        queue_num=(2 * mt + half) % 4,
                        sbuf_tokens_per_rank=P,
                        sbuf_free_dim_per_rank=2 * D,
                    )

            def emit_conv(b, g):
                st = state[b]
                xhi = st["xhi"]
                gstart, gtiles = groups[g]
                gtok = gtiles * P
                g0 = gstart * P
                gv = st["g"].pop(g)[:, 0, : gtiles * 1024].rearrange(
                    "d (t u k r) -> d t u k r", t=gtiles, u=8, k=8, r=16
                )
                for h in range(2):
                    cp = psum_conv_pool.tile([P, CHUNK], f32, tag="pconv")
                    # k = 0: self columns from the fp16 x2 copy
                    nc.tensor.matmul(
                        cp[:, :gtok],
                        lhsT=wt_v[:, 0, h, :],
                        rhs=xhi[:, g0 : g0 + gtok],
                        start=True, stop=False,
                    )
                    for k in range(1, K):
                        nc.tensor.matmul(
                            cp[:, :gtok],
                            lhsT=wt_v[:, k, h, :],
                            rhs=gv[:, :, :, k - 1, :],
                            start=False, stop=(k == K - 1),
                        )
                    o_s = outs_pool.tile([P, CHUNK], f32, tag="outs")
                    nc.scalar.activation(
                        o_s[:, :gtok], cp[:, :gtok],
                        mybir.ActivationFunctionType.Relu,
                        bias=bias_s[:, h : h + 1],
                    )
                    nc.scalar.dma_start(
                        out_d.ap()[b, h * P : (h + 1) * P, g0 : g0 + gtok],
                        o_s[:, :gtok],
                    )
                if g == len(groups) - 1:
                    state.pop(b, None)

            # ---- software-pipelined emission over flat tile slots ----
            group_end = {gs + gn - 1: gi for gi, (gs, gn) in enumerate(groups)}

            emit_bstart(0)
            for s in range(T + CONV_DELAY + 1):
                if s + BSTART_LEAD < T and (s + BSTART_LEAD) % nt == 0:
                    emit_bstart((s + BSTART_LEAD) // nt)
                if s < T:
                    b, mt = divmod(s, nt)
                    emit_scores(b, mt)
                    emit_topk(b, mt)
                    emit_shuffle(b, mt)
                t_g = s - GATHER_LAG
                if 0 <= t_g < T:
                    b, mt = divmod(t_g, nt)
                    emit_gathers(b, mt)
                t_c = s - CONV_DELAY
                if 0 <= t_c < T:
                    b, mt = divmod(t_c, nt)
                    if mt in group_end:
                        emit_conv(b, group_end[mt])

    nc.compile()
    _cache[key] = nc
    return nc, None


def _host_inputs(x, conv_w, conv_b):
    """Per-batch device inputs + shared per-core constants."""
    x = np.ascontiguousarray(x, dtype=np.float32)
    b = x.shape[0]
    x1 = (
        x.reshape(b, C_IN, H // S, S, W // S, S)
        .transpose(0, 1, 3, 5, 2, 4)
        .reshape(b, D, N)
    )
    xhi = x1.astype(np.float16)
    # SBUF gather table: token t = r*128 + p lives at [part p, r*128 : (r+1)*128]
    x2t = np.ascontiguousarray(
        xhi.transpose(0, 2, 1)
        .reshape(b, N // 128, 128, D)
        .transpose(0, 2, 1, 3)
        .reshape(b, 128, N // 128 * D)
    )
    v = -0.5 * np.einsum("bdn,bdn->bn", x1.astype(np.float64), x1.astype(np.float64))
    nhi = v.astype(np.float16)
    nlo = (v - nhi.astype(np.float64)).astype(np.float16)
    nsq2 = np.stack([nhi, nlo], axis=1)                           # [B, 2, N]
    per_batch = dict(x2=x1, x2t=x2t, xhi=np.ascontiguousarray(xhi), nsq2=nsq2)

    wt = np.ascontiguousarray(
        conv_w.reshape(2, P, D, K).transpose(2, 3, 0, 1), dtype=np.float16
    )  # [D, K, 2, P]
    bias = np.ascontiguousarray(
        conv_b.reshape(2, P).transpose(1, 0), dtype=np.float32
    )  # [P, 2]
    ones2 = np.ones((2, P), dtype=np.float16)
    negbig = (NEGBIG * np.eye(P)).astype(np.float16)
    ident = np.eye(P, dtype=np.float16)
    consts = dict(wt=wt, bias=bias, ones2=ones2, negbig=negbig, ident=ident)
    return per_batch, consts


def kernel(x, conv_w, conv_b):
    nc, _ = _build_kernel()
    per_batch, consts = _host_inputs(x, conv_w, conv_b)
    in_maps = []
    for c in range(NCORES):
        m = dict(consts)
        for k, v in per_batch.items():
            m[k] = np.ascontiguousarray(v[c * BPC : (c + 1) * BPC])
        in_maps.append(m)
    res = bass_utils.run_bass_kernel_spmd(nc, in_maps, core_ids=list(range(NCORES)))
    outs = np.concatenate([r["out"] for r in res.results], axis=0)  # [B, 256, N]
    # pixel shuffle back: channel dim = (co, sy, sx); token = (h, w)
    o = outs.reshape(B, C_OUT, S, S, H // S, W // S)
    o = o.transpose(0, 1, 4, 2, 5, 3).reshape(B, C_OUT, H, W)
    return np.ascontiguousarray(o, dtype=np.float32)


# revision 28
# speedup vs baseline: 1.1525x; 1.0084x over previous
"""Trainium2 Bass kernel for Conv2d_NN (k-NN gather + grouped conv1d).

Shapes (hardcoded): x (32, 32, 96, 96) f32, conv_w (256, 128, 9) f32,
conv_b (256,) f32 -> out (32, 64, 96, 96) f32.

Strategy: data-parallel over batch across 8 NeuronCores (4 batches/core).
Per batch on device (tokens N=2304, features D=128 after host pixel-unshuffle):
  - scores = x2^T @ x2 - 0.5*||x_j||^2 in fp32 on PE (fp32 matmul streams at
    ~2 cyc/col on HW), PSUM-chunked [128,512]x5; the j-dependent nsq term is
    one 2-row fp16 matmul per chunk (host-prepared exact hi+lo split); self
    excluded with a -16384 fp16 diag matmul.  Ranking needs fp32-exact
    scores: fp16/bf16 storage or single-fp16-product dots flip neighbors
    and push rel err to 5e-2..1e-1 (measured on host).
  - ACT evacuates PSUM chunks to an fp32 scores row-block; DVE max8 /
    find_index8 give the top-8 neighbor indices per token.
  - a 2-hop DMA shuffle (SBUF->DRAM->SBUF broadcast) rewraps the [128,8]
    index tile into the 16-partition-wrapped layout, k-major per 512-token
    conv group.
  - dma_gather (SWDGE, transpose mode) pulls neighbor token rows from a
    host-prepared fp16 [N,128] DRAM table straight into conv-rhs layout
    [128 feat, 8*512 tok].  This replaces the baseline's gpsimd ap_gather,
    which ran ~250us per call on the DSP cores and serialized the whole
    kernel (2.15ms); descriptor-generated DMA does the same gather in ~10us
    and overlaps with compute.
  - conv1d = 9 accumulating 128x128 fp16 matmuls per output half (k=0 rhs is
    the fp16 x2 copy, k=1..8 slices of the gathered buffer); ACT adds bias +
    ReLU; DMA writes (b, 256, N) fp32.  fp16 conv adds ~3e-4 rel err.
Host does pixel-unshuffle/shuffle and all dtype prep (fp16 table, nsq
hi/lo rows, fp16 weights).
"""

import sys

for _p in ("/opt/trn_rl_repo",):
    if _p not in sys.path:
        sys.path.insert(0, _p)

import numpy as np

import concourse.bass as bass
import concourse.mybir as mybir
import concourse.tile as tile
from concourse import bacc, bass_utils

# Problem constants
B, C_IN, C_OUT, H, W = 32, 32, 64, 96, 96
S = 2
K = 9
D = C_IN * S * S            # 128
D_OUT = C_OUT * S * S       # 256
N = (H // S) * (W // S)     # 2304
NCORES = 8
BPC = B // NCORES           # 4 batches per core

P = 128                     # partitions / m-tile size
NT = N // P                 # 18 m-tiles
CHUNK = 512                 # psum bank = 512 f32; conv group chunk
SCHUNK = 512                # scores psum chunk = 1 bank (matmul cannot cross banks)
CHUNKS = [(c, min(SCHUNK, N - c)) for c in range(0, N, SCHUNK)]  # 4x512 + 256
NEGBIG = -16384.0           # fp16-exact, dominates any real score
GROUP_TILES = 4             # m-tiles per conv group (512 tokens)

# pipeline lags (in tile slots)
GATHER_LAG = 1              # gather emitted this many slots after its group ends
BSTART_LEAD = 6             # batch-start work emitted this many slots early
CONV_DELAY = 14             # conv emitted this many slots after its group ends

_cache = {}


def _build_kernel(bpc=BPC, nt=NT):
    key = ("nc", bpc, nt)
    if key in _cache:
        return _cache[key], None

    nc = bacc.Bacc(
        "TRN2", target_bir_lowering=False, debug=False, num_swdge_queues=4
    )

    f32 = mybir.dt.float32
    fp16 = mybir.dt.float16
    u16 = mybir.dt.uint16
    i16 = mybir.dt.int16

    n_tok = nt * P

    # groups per batch: (start_tile, n_tiles)
    groups = []
    mt = 0
    while mt < nt:
        gt = min(GROUP_TILES, nt - mt)
        groups.append((mt, gt))
        mt += gt
    widx_w = nt * 64                             # 64 wrapped cols per tile

    # I/O
    x2_d = nc.dram_tensor("x2", [bpc, D, n_tok], f32, kind="ExternalInput")
    x2t_d = nc.dram_tensor("x2t", [bpc, P, (n_tok // P) * P], fp16, kind="ExternalInput")
    xhi_d = nc.dram_tensor("xhi", [bpc, D, n_tok], fp16, kind="ExternalInput")
    nsq2_d = nc.dram_tensor("nsq2", [bpc, 2, n_tok], fp16, kind="ExternalInput")
    wt_d = nc.dram_tensor("wt", [D, K, 2, P], fp16, kind="ExternalInput")
    bias_d = nc.dram_tensor("bias", [P, 2], f32, kind="ExternalInput")
    ones2_d = nc.dram_tensor("ones2", [2, P], fp16, kind="ExternalInput")
    negbig_d = nc.dram_tensor("negbig", [P, P], fp16, kind="ExternalInput")
    ident_d = nc.dram_tensor("ident", [P, P], fp16, kind="ExternalInput")
    out_d = nc.dram_tensor("out", [bpc, D_OUT, n_tok], f32, kind="ExternalOutput")

    T = bpc * nt                                 # total tile slots

    with tile.TileContext(nc) as tc:
        import contextlib

        with contextlib.ExitStack() as ctx:
            const_pool = ctx.enter_context(tc.tile_pool(name="consts", bufs=1))
            x2_pool = ctx.enter_context(tc.tile_pool(name="x2", bufs=2))
            xhi_pool = ctx.enter_context(tc.tile_pool(name="xhi", bufs=2))
            tbl_pool = ctx.enter_context(tc.tile_pool(name="tbl", bufs=2))
            nsq_pool = ctx.enter_context(tc.tile_pool(name="nsq", bufs=2))
            scores_pool = ctx.enter_context(tc.tile_pool(name="scores", bufs=4))
            mx_pool = ctx.enter_context(tc.tile_pool(name="mx", bufs=8))
            widx_pool = ctx.enter_context(tc.tile_pool(name="widx", bufs=2))
            g_pool = ctx.enter_context(tc.tile_pool(name="g", bufs=6))
            outs_pool = ctx.enter_context(tc.tile_pool(name="outs", bufs=3))
            psum_sc_pool = ctx.enter_context(
                tc.tile_pool(name="psums", bufs=5, space="PSUM")
            )
            psum_conv_pool = ctx.enter_context(
                tc.tile_pool(name="psumc", bufs=2, space="PSUM")
            )
            dram_pool = ctx.enter_context(
                tc.tile_pool(name="stage", bufs=8, space="DRAM")
            )

            # constants, loaded once
            wt_s = const_pool.tile([D, K * 2 * P], fp16, tag="wt")
            nc.sync.dma_start(wt_s[:], wt_d.ap().rearrange("d k h c -> d (k h c)"))
            wt_v = wt_s[:].rearrange("d (k h c) -> d k h c", k=K, h=2, c=P)
            bias_s = const_pool.tile([P, 2], f32, tag="bias")
            nc.sync.dma_start(bias_s[:], bias_d.ap())
            ones2_s = const_pool.tile([2, P], fp16, tag="ones2")
            nc.sync.dma_start(ones2_s[:], ones2_d.ap())
            negbig_s = const_pool.tile([P, P], fp16, tag="negbig")
            nc.sync.dma_start(negbig_s[:], negbig_d.ap())
            ident_s = const_pool.tile([P, P], fp16, tag="ident")
            nc.sync.dma_start(ident_s[:], ident_d.ap())

            # per-batch state (rotating pool tiles), keyed by batch
            state = {}

            def emit_bstart(b):
                x2 = x2_pool.tile([D, n_tok], f32, tag="x2")
                nc.scalar.dma_start(x2[:], x2_d.ap()[b])
                xhi = xhi_pool.tile([D, n_tok], fp16, tag="xhi")
                nc.scalar.dma_start(xhi[:], xhi_d.ap()[b])
                nsq2 = nsq_pool.tile([2, n_tok], fp16, tag="nsq2")
                nc.scalar.dma_start(nsq2[:], nsq2_d.ap()[b])
                tbl = tbl_pool.tile([P, n_tok], fp16, tag="tbl")
                nc.scalar.dma_start(tbl[:], x2t_d.ap()[b])
                widx = widx_pool.tile([P, widx_w], i16, tag="widx")
                state[b] = dict(
                    x2=x2, xhi=xhi, nsq2=nsq2, tbl=tbl, widx=widx, g={}
                )

            def emit_scores(b, mt):
                st = state[b]
                x2 = st["x2"]
                nsq2 = st["nsq2"]
                m0 = mt * P
                dc = m0 // SCHUNK
                pscs = []
                for ci, (c0, w) in enumerate(CHUNKS):
                    psc = psum_sc_pool.tile([P, SCHUNK], f32, tag="psc")
                    pscs.append(psc)
                    nc.tensor.matmul(
                        psc[:, :w],
                        lhsT=x2[:, m0 : m0 + P],
                        rhs=x2[:, c0 : c0 + w],
                        start=True, stop=False,
                    )
                # self-exclusion: scores[p, m0+p] += NEGBIG
                d0 = m0 - dc * SCHUNK
                nc.tensor.matmul(
                    pscs[dc][:, d0 : d0 + P],
                    lhsT=negbig_s[:],
                    rhs=ident_s[:],
                    start=False, stop=False,
                )
                # j-dependent -0.5*nsq[j] as one 2-row fp16 matmul per chunk
                for ci, (c0, w) in enumerate(CHUNKS):
                    nc.tensor.matmul(
                        pscs[ci][:, :w],
                        lhsT=ones2_s[:],
                        rhs=nsq2[:, c0 : c0 + w],
                        start=False, stop=True,
                    )
                scores = scores_pool.tile([P, n_tok], f32, tag="scores")
                for ci, (c0, w) in enumerate(CHUNKS):
                    nc.scalar.copy(scores[:, c0 : c0 + w], pscs[ci][:, :w])
                st["scores"] = st.get("scores", {})
                st["scores"][mt] = scores

            def emit_topk(b, mt):
                st = state[b]
                scores = st["scores"].pop(mt)
                mx8 = mx_pool.tile([P, 8], f32, tag="mx8")
                nc.vector.max(out=mx8[:], in_=scores[:])
                midx = mx_pool.tile([P, 8], u16, tag="midx")
                nc.vector.max_index(midx[:], mx8[:], scores[:])
                st["midx"] = st.get("midx", {})
                st["midx"][mt] = midx

            def emit_shuffle(b, mt):
                st = state[b]
                midx = st["midx"].pop(mt)
                wg = st["widx"]
                # hop 1: midx [128,8] -> staging[(r*8 + u)*8 + k] (DRAM);
                # k innermost keeps both hops at 16B-contiguous DMA runs
                stage_t = dram_pool.tile([1, 1024], u16, tag="stage")
                st_dst = stage_t[:].rearrange(
                    "a (r u k) -> a u r k", r=16, u=8, k=8
                ).squeeze(0)
                nc.sync.dma_start(st_dst, midx[:])
                # hop 2: widx[16c+r, mt*64 + c2] = staging[r*64 + c2]
                st_src = (
                    stage_t[:]
                    .rearrange("a (r c2) -> a r c2", r=16, c2=64)
                    .unsqueeze(1)
                    .broadcast_to([1, 8, 16, 64])
                    .bitcast(i16)
                    .squeeze(0)
                )
                nc.sync.dma_start(wg[:, mt * 64 : (mt + 1) * 64], st_src)

            def emit_gathers(b, mt):
                # two 512-idx gathers per tile (1024-descriptor SWDGE carveout
                # limit per instruction), round-robin over the 4 SWDGE queues.
                # idx stream position i = ((u*8 + k)*16 + r) within the tile,
                # i.e. token-half-major; conv rhs un-permutes with a 4D view.
                st = state[b]
                if mt % GROUP_TILES == 0 or mt == groups[-1][0]:
                    g = [gi for gi, (gs, gn) in enumerate(groups) if gs == mt][0]
                    gg = g_pool.tile(
                        [P, 1, GROUP_TILES * 1024], fp16, tag="g"
                    )
                    st["g"][g] = gg
                g = [gi for gi, (gs, gn) in enumerate(groups)
                     if gs <= mt < gs + gn][0]
                tloc = mt - groups[g][0]
                gg = st["g"][g]
                for half in range(2):
                    o = tloc * 1024 + half * 512
                    nc.gpsimd.dma_gather(
                        gg[:, :, o : o + 512],
                        st["tbl"][:],
                        st["widx"][:, mt * 64 + half * 32 : mt * 64 + half * 32 + 32],
                        512,
                        512,
                        D,
                        transpose=True,
                        queue_num=(2 * mt + half) % 4,
                        sbuf_tokens_per_rank=P,
                        sbuf_free_dim_per_rank=2 * D,
                    )

            def emit_conv(b, g):
                st = state[b]
                xhi = st["xhi"]
                gstart, gtiles = groups[g]
                gtok = gtiles * P
                g0 = gstart * P
                gv = st["g"].pop(g)[:, 0, : gtiles * 1024].rearrange(
                    "d (t u k r) -> d t u k r", t=gtiles, u=8, k=8, r=16
                )
                for h in range(2):
                    cp = psum_conv_pool.tile([P, CHUNK], f32, tag="pconv")
                    # k = 0: self columns from the fp16 x2 copy
                    nc.tensor.matmul(
                        cp[:, :gtok],
                        lhsT=wt_v[:, 0, h, :],
                        rhs=xhi[:, g0 : g0 + gtok],
                        start=True, stop=False,
                    )
                    for k in range(1, K):
                        nc.tensor.matmul(
                            cp[:, :gtok],
                            lhsT=wt_v[:, k, h, :],
                            rhs=gv[:, :, :, k - 1, :],
                            start=False, stop=(k == K - 1),
                        )
                    o_s = outs_pool.tile([P, CHUNK], f32, tag="outs")
                    nc.scalar.activation(
                        o_s[:, :gtok], cp[:, :gtok],
                        mybir.ActivationFunctionType.Relu,
                        bias=bias_s[:, h : h + 1],
                    )
                    nc.scalar.dma_start(
                        out_d.ap()[b, h * P : (h + 1) * P, g0 : g0 + gtok],
                        o_s[:, :gtok],
                    )
                if g == len(groups) - 1:
                    state.pop(b, None)

            # ---- software-pipelined emission over flat tile slots ----
            group_end = {gs + gn - 1: gi for gi, (gs, gn) in enumerate(groups)}

            emit_bstart(0)
            for s in range(T + CONV_DELAY + 1):
                if s + BSTART_LEAD < T and (s + BSTART_LEAD) % nt == 0:
                    emit_bstart((s + BSTART_LEAD) // nt)
                if s < T:
                    b, mt = divmod(s, nt)
                    emit_scores(b, mt)
                    emit_topk(b, mt)
                    emit_shuffle(b, mt)
                t_g = s - GATHER_LAG
                if 0 <= t_g < T:
                    b, mt = divmod(t_g, nt)
                    emit_gathers(b, mt)
                t_c = s - CONV_DELAY
                if 0 <= t_c < T:
                    b, mt = divmod(t_c, nt)
                    if mt in group_end:
                        emit_conv(b, group_end[mt])

    nc.compile()
    _cache[key] = nc
    return nc, None


def _host_inputs(x, conv_w, conv_b):
    """Per-batch device inputs + shared per-core constants."""
    x = np.ascontiguousarray(x, dtype=np.float32)
    b = x.shape[0]
    x1 = (
        x.reshape(b, C_IN, H // S, S, W // S, S)
        .transpose(0, 1, 3, 5, 2, 4)
        .reshape(b, D, N)
    )
    xhi = x1.astype(np.float16)
    # SBUF gather table: token t = r*128 + p lives at [part p, r*128 : (r+1)*128]
    x2t = np.ascontiguousarray(
        xhi.transpose(0, 2, 1)
        .reshape(b, N // 128, 128, D)
        .transpose(0, 2, 1, 3)
        .reshape(b, 128, N // 128 * D)
    )
    v = -0.5 * np.einsum("bdn,bdn->bn", x1.astype(np.float64), x1.astype(np.float64))
    nhi = v.astype(np.float16)
    nlo = (v - nhi.astype(np.float64)).astype(np.float16)
    nsq2 = np.stack([nhi, nlo], axis=1)                           # [B, 2, N]
    per_batch = dict(x2=x1, x2t=x2t, xhi=np.ascontiguousarray(xhi), nsq2=nsq2)

    wt = np.ascontiguousarray(
        conv_w.reshape(2, P, D, K).transpose(2, 3, 0, 1), dtype=np.float16
    )  # [D, K, 2, P]
    bias = np.ascontiguousarray(
        conv_b.reshape(2, P).transpose(1, 0), dtype=np.float32
    )  # [P, 2]
    ones2 = np.ones((2, P), dtype=np.float16)
    negbig = (NEGBIG * np.eye(P)).astype(np.float16)
    ident = np.eye(P, dtype=np.float16)
    consts = dict(wt=wt, bias=bias, ones2=ones2, negbig=negbig, ident=ident)
    return per_batch, consts


def kernel(x, conv_w, conv_b):
    nc, _ = _build_kernel()
    per_batch, consts = _host_inputs(x, conv_w, conv_b)
    in_maps = []
    for c in range(NCORES):
        m = dict(consts)
        for k, v in per_batch.items():
            m[k] = np.ascontiguousarray(v[c * BPC : (c + 1) * BPC])
        in_maps.append(m)
    res = bass_utils.run_bass_kernel_spmd(nc, in_maps, core_ids=list(range(NCORES)))
    outs = np.concatenate([r["out"] for r in res.results], axis=0)  # [B, 256, N]
    # pixel shuffle back: channel dim = (co, sy, sx); token = (h, w)
    o = outs.reshape(B, C_OUT, S, S, H // S, W // S)
    o = o.transpose(0, 1, 4, 2, 5, 3).reshape(B, C_OUT, H, W)
    return np.ascontiguousarray(o, dtype=np.float32)
